# revision 12
# baseline (speedup 1.0000x reference)
"""Trainium2 Bass kernel for the sparse-attention module.

Reference computation (per batch element b):
    q = wq @ x + bq ; k = wk @ x + bk ; v = wv @ x + bv        # [S, N]
    att[i, j] = softmax_j( sum_s k[s, i] * q[s, j] )           # [N, N]
    v2 = v @ att                                               # [S, N]
    out = coef * (wa @ v2 + ba) + x                            # [C, N]
returns (out, att).

Sharding: pure data parallel over batch — B=8 batch elements, one per
NeuronCore. Params are tiny and replicated (pre-transposed on host into
PE-friendly layouts).

Per-core dataflow (C=512, N=4096, S=64):
  phase A: stream x in 512-column tiles, project q/k/v on PE (f32r
           matmuls -> bf16), DMA-xbar-transpose v -> vT (bf16).
           qkv PSUM tiles share one pool with phase B's att quarters so
           the PE instruction stream never pauses long enough for the
           HAM clock gate to re-throttle.
  phase B: per 128-row tile of att: PE k^T q (bf16) into double-buffered
           [128,1024] PSUM quarters, ACT exp -> bf16 SBUF, GpSimd row
           sums, DVE 1/sum scale, DMA att rows out as bf16 (host widens
           to f32), PE v2 accumulation (bf16, col-paired into both PSUM
           partition halves)
  phase C: PE sa = (coef*wa) @ v2_aug (+coef*ba via ones row) + x via
           identity matmul, ACT evacuates PSUM, DMA out
"""

import numpy as np

B, C, H, W = 8, 512, 64, 64
N = H * W          # 4096 tokens
S = C // 8         # 64   small channels
CK = C // 128      # 4    c-chunks (partition blocks of x / out)
NT = N // 128      # 32   n-tiles (att row blocks)
MC = N // 512      # 8    m-chunks (512-wide matmul free dim)

_CACHED = None


def _build():
    import concourse.tile as tile
    from concourse import bacc, mybir

    F32 = mybir.dt.float32
    F32R = mybir.dt.float32r
    BF16 = mybir.dt.bfloat16
    EXP = mybir.ActivationFunctionType.Exp
    COPY = mybir.ActivationFunctionType.Copy
    AX = mybir.AxisListType.X

    nc = bacc.Bacc("TRN2", target_bir_lowering=False, debug=False)

    x_d = nc.dram_tensor("x", [C, N], F32R, kind="ExternalInput")
    wqT_d = nc.dram_tensor("wqT", [C, S], F32R, kind="ExternalInput")
    wkT_d = nc.dram_tensor("wkT", [C, S], F32R, kind="ExternalInput")
    wvT_d = nc.dram_tensor("wvT", [C, S], F32R, kind="ExternalInput")
    bq_d = nc.dram_tensor("bq", [S, 1], F32, kind="ExternalInput")
    bk_d = nc.dram_tensor("bk", [S, 1], F32, kind="ExternalInput")
    bv_d = nc.dram_tensor("bv", [S, 1], F32, kind="ExternalInput")
    waT_d = nc.dram_tensor("waT", [S, C], F32R, kind="ExternalInput")
    ba_row_d = nc.dram_tensor("ba_row", [1, C], F32R, kind="ExternalInput")
    coef_d = nc.dram_tensor("coefb", [128, 1], F32, kind="ExternalInput")
    id128_d = nc.dram_tensor("ident128", [128, 128], F32R, kind="ExternalInput")
    id64_d = nc.dram_tensor("ident64", [S, S], BF16, kind="ExternalInput")
    ones_d = nc.dram_tensor("ones_row", [1, N], F32R, kind="ExternalInput")

    out_d = nc.dram_tensor("out", [C, N], F32, kind="ExternalOutput")
    att_d = nc.dram_tensor("att", [N, N], BF16, kind="ExternalOutput")

    with tile.TileContext(nc) as tc:
        with (
            tc.tile_pool(name="persist", bufs=1) as pp,
            tc.tile_pool(name="attp", bufs=4) as attp,
            tc.tile_pool(name="outp", bufs=2) as outp,
            tc.tile_pool(name="stats", bufs=8) as statp,
        ):
            # fine-grained tiles so consumers wait only on what they need
            xt = {}
            for j in range(MC):
                for kk in range(CK):
                    xt[(j, kk)] = pp.tile([128, 512], F32R, name=f"x_{j}_{kk}")
            qt = [pp.tile([S, 512], BF16, name=f"q_{j}") for j in range(MC)]
            kt = [pp.tile([S, 512], BF16, name=f"k_{j}") for j in range(MC)]
            vt = [pp.tile([S, 512], BF16, name=f"v_{j}") for j in range(MC)]
            vTt = [pp.tile([128, 4, S], BF16, name=f"vT_{j}") for j in range(MC)]
            v2_aug = pp.tile([S + 1, N], F32R)
            v2h_sb = pp.tile([128, N // 2], F32R)
            wqT_sb = pp.tile([128, CK, S], F32R)
            wkT_sb = pp.tile([128, CK, S], F32R)
            wvT_sb = pp.tile([128, CK, S], F32R)
            waT_aug = pp.tile([S + 1, C], F32R)
            bq_sb = pp.tile([S, 1], F32)
            bk_sb = pp.tile([S, 1], F32)
            bv_sb = pp.tile([S, 1], F32)
            coef_sb = pp.tile([128, 1], F32)
            id128_sb = pp.tile([128, 128], F32R)
            id64_sb = pp.tile([S, S], BF16)

            x_re = x_d.ap().rearrange("(kk p) n -> p kk n", p=128)
            for kk in range(CK):
                nc.sync.dma_start(xt[(0, kk)][:], x_re[:, kk, 0:512])
            for w_sb, w_d in ((wqT_sb, wqT_d), (wkT_sb, wkT_d)):
                nc.sync.dma_start(
                    w_sb[:], w_d.ap().rearrange("(kk p) s -> p kk s", p=128)
                )
            nc.sync.dma_start(bq_sb[:], bq_d.ap())
            nc.sync.dma_start(bk_sb[:], bk_d.ap())
            for j in range(1, MC):
                for kk in range(CK):
                    nc.sync.dma_start(
                        xt[(j, kk)][:], x_re[:, kk, 512 * j : 512 * (j + 1)]
                    )
            nc.sync.dma_start(
                wvT_sb[:], wvT_d.ap().rearrange("(kk p) s -> p kk s", p=128)
            )
            nc.sync.dma_start(bv_sb[:], bv_d.ap())
            nc.sync.dma_start(waT_aug[0:S, :], waT_d.ap())
            nc.sync.dma_start(waT_aug[S : S + 1, :], ba_row_d.ap())
            nc.sync.dma_start(coef_sb[:], coef_d.ap())
            nc.sync.dma_start(id128_sb[:], id128_d.ap())
            nc.sync.dma_start(id64_sb[:], id64_d.ap())

            # sa weights scaled by coef on device; row S carries coef*ba
            nc.vector.tensor_scalar_mul(
                waT_aug[:], waT_aug[:], coef_sb[0 : S + 1, :]
            )
            nc.sync.dma_start(v2_aug[S : S + 1, :], ones_d.ap())

            # one shared pool for qkv psum chunks AND att psum quarters:
            # the PE stream flows from projections straight into attention
            # with no pool barrier in between
            with (
                tc.tile_pool(name="psMain", bufs=2, space="PSUM") as psM,
                tc.tile_pool(name="psV", bufs=1, space="PSUM") as psV,
            ):
                # ---------- phase A: q/k projections (v is folded into
                # the first phase-B iterations)
                def project(w_sb, b_sb, dst, j):
                    ps = psM.tile([S, 512], F32, tag="mm", name="ps_prj")
                    for kk in range(CK):
                        nc.tensor.matmul(
                            ps[:],
                            w_sb[:, kk, :],
                            xt[(j, kk)][:],
                            start=(kk == 0),
                            stop=(kk == CK - 1),
                        )
                    nc.vector.tensor_scalar_add(dst[j][:], ps[:], b_sb[:])

                for j in range(MC):
                    project(wqT_sb, bq_sb, qt, j)
                    project(wkT_sb, bk_sb, kt, j)

                def emit_v_chunk(j):
                    project(wvT_sb, bv_sb, vt, j)
                    for i in range(4 * j, 4 * j + 4):
                        pst = psM.tile([128, S], BF16, tag="mm", name="ps_tp")
                        nc.tensor.transpose(
                            pst[:],
                            vt[j][:, 128 * (i % 4) : 128 * (i % 4 + 1)],
                            id64_sb[:],
                        )
                        nc.vector.tensor_copy(vTt[j][:, i % 4, :], pst[:])

                # ---------- phase B: attention rows + v2 accumulation
                v2ps = psV.tile([128, 4 * 512], F32)

                def emit_v2(i, asb_i, jlist):
                    # interleave the two psum col-groups so the pairs run
                    # concurrently on the PE quadrants
                    for j in jlist:
                        rb = 64 * (j // 4)
                        nc.tensor.matmul(
                            v2ps[rb : rb + 64, 512 * (j % 4) : 512 * (j % 4) + 512],
                            vTt[i // 4][:, i % 4, :],
                            asb_i[:, 512 * j : 512 * (j + 1)],
                            start=(i == 0),
                            stop=(i == NT - 1),
                            tile_position=(0, rb),
                            skip_group_check=True,
                        )

                prev = None  # (i, asb) whose v2 matmuls are still pending
                for i in range(NT):
                    if i < 2 * MC and i % 2 == 0:
                        emit_v_chunk(i // 2)
                    asb = attp.tile([128, N], BF16)
                    st = statp.tile([128, 8], F32)
                    kt_i = kt[i // 4]
                    ko = 128 * (i % 4)
                    for qq in range(4):
                        aps = psM.tile([128, 1024], F32, tag="mm")
                        for j in range(2):
                            nc.tensor.matmul(
                                aps[:, 512 * j : 512 * (j + 1)],
                                kt_i[:, ko : ko + 128],
                                qt[2 * qq + j][:],
                                start=True,
                                stop=True,
                            )
                        if qq < 2:
                            nc.scalar.activation(
                                asb[:, 1024 * qq : 1024 * (qq + 1)],
                                aps[:],
                                EXP,
                                accum_out=st[:, qq : qq + 1],
                            )
                        else:
                            nc.scalar.activation(
                                asb[:, 1024 * qq : 1024 * (qq + 1)],
                                aps[:],
                                EXP,
                            )
                            nc.vector.reduce_sum(
                                st[:, qq : qq + 1],
                                asb[:, 1024 * qq : 1024 * (qq + 1)],
                                axis=AX,
                            )
                        if qq == 1 and prev is not None:
                            emit_v2(prev[0], prev[1], [0, 4, 1, 5])
                        if qq == 3 and prev is not None:
                            emit_v2(prev[0], prev[1], [2, 6])
                    nc.vector.reduce_sum(st[:, 4:5], st[:, 0:4], axis=AX)
                    nc.vector.reciprocal(st[:, 5:6], st[:, 4:5])
                    nc.vector.tensor_scalar_mul(asb[:], asb[:], st[:, 5:6])
                    nc.sync.dma_start(att_d.ap()[128 * i : 128 * (i + 1), :], asb[:])
                    if prev is not None:
                        emit_v2(prev[0], prev[1], [3, 7])
                    prev = (i, asb)
                emit_v2(prev[0], prev[1], [0, 4, 1, 5, 2, 6, 3, 7])

                # v2ps rows 0-63 hold m-chunks 0-3; rows 64-127 hold 4-7.
                # Everything must land on partitions 0-64 for the sa matmuls:
                # lower half straight via DVE, upper half DVE->SBUF then a
                # cross-partition SBUF->SBUF DMA.
                nc.vector.tensor_copy(v2_aug[0:S, 0 : N // 2], v2ps[0:64, :])
                nc.vector.tensor_copy(v2h_sb[64:128, :], v2ps[64:128, :])
                nc.sync.dma_start(v2_aug[0:S, N // 2 : N], v2h_sb[64:128, :])

            # ---------- phase C: out = coef*(wa@v2 + ba) + x, all on PE
            with tc.tile_pool(name="psD", bufs=2, space="PSUM") as psD:
                for kk in range(CK):
                    for h in range(2):
                        sps = psD.tile([128, 2048], F32)
                        for j in range(4):
                            jj = 4 * h + j
                            nc.tensor.matmul(
                                sps[:, 512 * j : 512 * (j + 1)],
                                waT_aug[:, 128 * kk : 128 * (kk + 1)],
                                v2_aug[:, 512 * jj : 512 * (jj + 1)],
                                start=True,
                                stop=False,
                                skip_group_check=True,
                            )
                            nc.tensor.matmul(
                                sps[:, 512 * j : 512 * (j + 1)],
                                id128_sb[:],
                                xt[(jj, kk)][:],
                                start=False,
                                stop=True,
                                skip_group_check=True,
                            )
                        osb = outp.tile([128, 2048], F32)
                        nc.scalar.activation(osb[:], sps[:], COPY)
                        nc.sync.dma_start(
                            out_d.ap()[
                                128 * kk : 128 * (kk + 1), 2048 * h : 2048 * (h + 1)
                            ],
                            osb[:],
                        )

    nc.compile()
    return nc


def _get_nc():
    global _CACHED
    if _CACHED is None:
        _CACHED = _build()
    return _CACHED


def make_in_maps(x, wq, bq, wk, bk, wv, bv, wa, ba, coef):
    import ml_dtypes

    x = np.asarray(x, dtype=np.float32)
    xf = np.ascontiguousarray(x.reshape(B, C, N))
    shared = {
        "wqT": np.ascontiguousarray(np.asarray(wq, np.float32).T),
        "wkT": np.ascontiguousarray(np.asarray(wk, np.float32).T),
        "wvT": np.ascontiguousarray(np.asarray(wv, np.float32).T),
        "bq": np.ascontiguousarray(np.asarray(bq, np.float32).reshape(S, 1)),
        "bk": np.ascontiguousarray(np.asarray(bk, np.float32).reshape(S, 1)),
        "bv": np.ascontiguousarray(np.asarray(bv, np.float32).reshape(S, 1)),
        "waT": np.ascontiguousarray(np.asarray(wa, np.float32).T),
        "ba_row": np.ascontiguousarray(np.asarray(ba, np.float32).reshape(1, C)),
        "coefb": np.full((128, 1), np.float32(np.asarray(coef).reshape(-1)[0])),
        "ident128": np.eye(128, dtype=np.float32),
        "ident64": np.eye(S, dtype=ml_dtypes.bfloat16),
        "ones_row": np.ones((1, N), dtype=np.float32),
    }
    return [dict(shared, x=np.ascontiguousarray(xf[b])) for b in range(B)]


def kernel(x, wq, bq, wk, bk, wv, bv, wa, ba, coef, **_unused):
    from concourse.bass_utils import run_bass_kernel_spmd

    nc = _get_nc()
    in_maps = make_in_maps(x, wq, bq, wk, bk, wv, bv, wa, ba, coef)
    res = run_bass_kernel_spmd(nc, in_maps, core_ids=list(range(B)))

    out = np.stack([res.results[b]["out"].reshape(C, H, W) for b in range(B)])
    att = np.stack(
        [res.results[b]["att"].astype(np.float32) for b in range(B)]
    )
    return out, att


# revision 13
# speedup vs baseline: 1.0048x; 1.0048x over previous
"""Trainium2 Bass kernel for the sparse-attention module.

Reference computation (per batch element b):
    q = wq @ x + bq ; k = wk @ x + bk ; v = wv @ x + bv        # [S, N]
    att[i, j] = softmax_j( sum_s k[s, i] * q[s, j] )           # [N, N]
    v2 = v @ att                                               # [S, N]
    out = coef * (wa @ v2 + ba) + x                            # [C, N]
returns (out, att).

Sharding: pure data parallel over batch — B=8 batch elements, one per
NeuronCore. Params are tiny and replicated (pre-transposed on host into
PE-friendly layouts).

Per-core dataflow (C=512, N=4096, S=64):
  phase A: stream x in 512-column tiles, project q/k/v on PE (f32r
           matmuls -> bf16), DMA-xbar-transpose v -> vT (bf16).
           qkv PSUM tiles share one pool with phase B's att quarters so
           the PE instruction stream never pauses long enough for the
           HAM clock gate to re-throttle.
  phase B: per 128-row tile of att: PE k^T q (bf16) into double-buffered
           [128,1024] PSUM quarters, ACT exp -> bf16 SBUF, GpSimd row
           sums, DVE 1/sum scale, DMA att rows out as bf16 (host widens
           to f32), PE v2 accumulation (bf16, col-paired into both PSUM
           partition halves)
  phase C: PE sa = (coef*wa) @ v2_aug (+coef*ba via ones row) + x via
           identity matmul, ACT evacuates PSUM, DMA out
"""

import numpy as np

B, C, H, W = 8, 512, 64, 64
N = H * W          # 4096 tokens
S = C // 8         # 64   small channels
CK = C // 128      # 4    c-chunks (partition blocks of x / out)
NT = N // 128      # 32   n-tiles (att row blocks)
MC = N // 512      # 8    m-chunks (512-wide matmul free dim)

_CACHED = None


def _build():
    import concourse.tile as tile
    from concourse import bacc, mybir

    F32 = mybir.dt.float32
    F32R = mybir.dt.float32r
    BF16 = mybir.dt.bfloat16
    EXP = mybir.ActivationFunctionType.Exp
    COPY = mybir.ActivationFunctionType.Copy
    AX = mybir.AxisListType.X

    nc = bacc.Bacc("TRN2", target_bir_lowering=False, debug=False)

    x_d = nc.dram_tensor("x", [C, N], F32R, kind="ExternalInput")
    wqT_d = nc.dram_tensor("wqT", [C, S], F32R, kind="ExternalInput")
    wkT_d = nc.dram_tensor("wkT", [C, S], F32R, kind="ExternalInput")
    wvT_d = nc.dram_tensor("wvT", [C, S], F32R, kind="ExternalInput")
    bq_d = nc.dram_tensor("bq", [S, 1], F32, kind="ExternalInput")
    bk_d = nc.dram_tensor("bk", [S, 1], F32, kind="ExternalInput")
    bv_d = nc.dram_tensor("bv", [S, 1], F32, kind="ExternalInput")
    waT_d = nc.dram_tensor("waT", [S, C], F32R, kind="ExternalInput")
    ba_row_d = nc.dram_tensor("ba_row", [1, C], F32R, kind="ExternalInput")
    coef_d = nc.dram_tensor("coefb", [128, 1], F32, kind="ExternalInput")
    id128_d = nc.dram_tensor("ident128", [128, 128], F32R, kind="ExternalInput")
    id64_d = nc.dram_tensor("ident64", [S, S], BF16, kind="ExternalInput")
    ones_d = nc.dram_tensor("ones_row", [1, N], F32R, kind="ExternalInput")

    out_d = nc.dram_tensor("out", [C, N], F32, kind="ExternalOutput")
    att_d = nc.dram_tensor("att", [N, N], BF16, kind="ExternalOutput")

    with tile.TileContext(nc) as tc:
        with (
            tc.tile_pool(name="persist", bufs=1) as pp,
            tc.tile_pool(name="attp", bufs=4) as attp,
            tc.tile_pool(name="outp", bufs=2) as outp,
            tc.tile_pool(name="stats", bufs=8) as statp,
        ):
            # fine-grained tiles so consumers wait only on what they need
            xt = {}
            for j in range(MC):
                for kk in range(CK):
                    xt[(j, kk)] = pp.tile([128, 512], F32R, name=f"x_{j}_{kk}")
            qt = [pp.tile([S, 512], BF16, name=f"q_{j}") for j in range(MC)]
            kt = [pp.tile([S, 512], BF16, name=f"k_{j}") for j in range(MC)]
            vt = [pp.tile([S, 512], BF16, name=f"v_{j}") for j in range(MC)]
            vTt = [pp.tile([128, 4, S], BF16, name=f"vT_{j}") for j in range(MC)]
            v2_aug = pp.tile([S + 1, N], F32R)
            v2h_sb = pp.tile([128, N // 2], F32R)
            wqT_sb = pp.tile([128, CK, S], F32R)
            wkT_sb = pp.tile([128, CK, S], F32R)
            wvT_sb = pp.tile([128, CK, S], F32R)
            waT_aug = pp.tile([S + 1, C], F32R)
            bq_sb = pp.tile([S, 1], F32)
            bk_sb = pp.tile([S, 1], F32)
            bv_sb = pp.tile([S, 1], F32)
            coef_sb = pp.tile([128, 1], F32)
            id128_sb = pp.tile([128, 128], F32R)
            id64_sb = pp.tile([S, S], BF16)

            x_re = x_d.ap().rearrange("(kk p) n -> p kk n", p=128)
            for kk in range(CK):
                nc.sync.dma_start(xt[(0, kk)][:], x_re[:, kk, 0:512])
            for w_sb, w_d in ((wqT_sb, wqT_d), (wkT_sb, wkT_d)):
                nc.sync.dma_start(
                    w_sb[:], w_d.ap().rearrange("(kk p) s -> p kk s", p=128)
                )
            nc.sync.dma_start(bq_sb[:], bq_d.ap())
            nc.sync.dma_start(bk_sb[:], bk_d.ap())
            for j in range(1, MC):
                for kk in range(CK):
                    nc.sync.dma_start(
                        xt[(j, kk)][:], x_re[:, kk, 512 * j : 512 * (j + 1)]
                    )
            nc.sync.dma_start(
                wvT_sb[:], wvT_d.ap().rearrange("(kk p) s -> p kk s", p=128)
            )
            nc.sync.dma_start(bv_sb[:], bv_d.ap())
            nc.sync.dma_start(waT_aug[0:S, :], waT_d.ap())
            nc.sync.dma_start(waT_aug[S : S + 1, :], ba_row_d.ap())
            nc.sync.dma_start(coef_sb[:], coef_d.ap())
            nc.sync.dma_start(id128_sb[:], id128_d.ap())
            nc.sync.dma_start(id64_sb[:], id64_d.ap())

            # sa weights scaled by coef on device; row S carries coef*ba
            nc.vector.tensor_scalar_mul(
                waT_aug[:], waT_aug[:], coef_sb[0 : S + 1, :]
            )
            nc.sync.dma_start(v2_aug[S : S + 1, :], ones_d.ap())

            # one shared pool for qkv psum chunks AND att psum quarters:
            # the PE stream flows from projections straight into attention
            # with no pool barrier in between
            with (
                tc.tile_pool(name="psMain", bufs=2, space="PSUM") as psM,
                tc.tile_pool(name="psV", bufs=1, space="PSUM") as psV,
            ):
                # ---------- phase A: q/k projections (v is folded into
                # the first phase-B iterations)
                def project(w_sb, b_sb, dst, j):
                    ps = psM.tile([S, 512], F32, tag="mm", name="ps_prj")
                    for kk in range(CK):
                        nc.tensor.matmul(
                            ps[:],
                            w_sb[:, kk, :],
                            xt[(j, kk)][:],
                            start=(kk == 0),
                            stop=(kk == CK - 1),
                        )
                    nc.vector.tensor_scalar_add(dst[j][:], ps[:], b_sb[:])

                for j in range(MC):
                    project(wqT_sb, bq_sb, qt, j)
                    project(wkT_sb, bk_sb, kt, j)

                def emit_v_chunk(j):
                    project(wvT_sb, bv_sb, vt, j)
                    for i in range(4 * j, 4 * j + 4):
                        pst = psM.tile([128, S], BF16, tag="mm", name="ps_tp")
                        nc.tensor.transpose(
                            pst[:],
                            vt[j][:, 128 * (i % 4) : 128 * (i % 4 + 1)],
                            id64_sb[:],
                        )
                        nc.vector.tensor_copy(vTt[j][:, i % 4, :], pst[:])

                # ---------- phase B: attention rows + v2 accumulation
                v2ps = psV.tile([128, 4 * 512], F32)

                def emit_v2(i, asb_i, jlist):
                    # interleave the two psum col-groups so the pairs run
                    # concurrently on the PE quadrants
                    for j in jlist:
                        rb = 64 * (j // 4)
                        nc.tensor.matmul(
                            v2ps[rb : rb + 64, 512 * (j % 4) : 512 * (j % 4) + 512],
                            vTt[i // 4][:, i % 4, :],
                            asb_i[:, 512 * j : 512 * (j + 1)],
                            start=(i == 0),
                            stop=(i == NT - 1),
                            tile_position=(0, rb),
                            skip_group_check=True,
                        )

                prev = None  # (i, asb) whose v2 matmuls are still pending
                v_sched = {0: [0, 1]}
                for j in range(2, MC):
                    v_sched.setdefault(4 * j - 3, []).append(j)
                for i in range(NT):
                    for j in v_sched.get(i, []):
                        emit_v_chunk(j)
                    asb = attp.tile([128, N], BF16)
                    st = statp.tile([128, 8], F32)
                    kt_i = kt[i // 4]
                    ko = 128 * (i % 4)
                    for qq in range(4):
                        aps = psM.tile([128, 1024], F32, tag="mm")
                        for j in range(2):
                            nc.tensor.matmul(
                                aps[:, 512 * j : 512 * (j + 1)],
                                kt_i[:, ko : ko + 128],
                                qt[2 * qq + j][:],
                                start=True,
                                stop=True,
                            )
                        nc.scalar.activation(
                            asb[:, 1024 * qq : 1024 * (qq + 1)],
                            aps[:],
                            EXP,
                            accum_out=st[:, qq : qq + 1],
                        )
                        if qq == 1 and prev is not None:
                            emit_v2(prev[0], prev[1], [0, 4, 1, 5])
                        if qq == 3 and prev is not None:
                            emit_v2(prev[0], prev[1], [2, 6])
                    nc.vector.reduce_sum(st[:, 4:5], st[:, 0:4], axis=AX)
                    nc.vector.reciprocal(st[:, 5:6], st[:, 4:5])
                    nc.vector.tensor_scalar_mul(asb[:], asb[:], st[:, 5:6])
                    nc.sync.dma_start(att_d.ap()[128 * i : 128 * (i + 1), :], asb[:])
                    if prev is not None:
                        emit_v2(prev[0], prev[1], [3, 7])
                    prev = (i, asb)
                emit_v2(prev[0], prev[1], [0, 4, 1, 5, 2, 6, 3, 7])

                # v2ps rows 0-63 hold m-chunks 0-3; rows 64-127 hold 4-7.
                # Everything must land on partitions 0-64 for the sa matmuls:
                # lower half straight via DVE, upper half DVE->SBUF then a
                # cross-partition SBUF->SBUF DMA.
                nc.vector.tensor_copy(v2_aug[0:S, 0 : N // 2], v2ps[0:64, :])
                nc.vector.tensor_copy(v2h_sb[64:128, :], v2ps[64:128, :])
                nc.sync.dma_start(v2_aug[0:S, N // 2 : N], v2h_sb[64:128, :])

            # ---------- phase C: out = coef*(wa@v2 + ba) + x, all on PE
            with tc.tile_pool(name="psD", bufs=2, space="PSUM") as psD:
                for kk in range(CK):
                    for h in range(2):
                        sps = psD.tile([128, 2048], F32)
                        for j in range(4):
                            jj = 4 * h + j
                            nc.tensor.matmul(
                                sps[:, 512 * j : 512 * (j + 1)],
                                waT_aug[:, 128 * kk : 128 * (kk + 1)],
                                v2_aug[:, 512 * jj : 512 * (jj + 1)],
                                start=True,
                                stop=False,
                                skip_group_check=True,
                            )
                            nc.tensor.matmul(
                                sps[:, 512 * j : 512 * (j + 1)],
                                id128_sb[:],
                                xt[(jj, kk)][:],
                                start=False,
                                stop=True,
                                skip_group_check=True,
                            )
                        osb = outp.tile([128, 2048], F32)
                        nc.scalar.activation(osb[:], sps[:], COPY)
                        nc.sync.dma_start(
                            out_d.ap()[
                                128 * kk : 128 * (kk + 1), 2048 * h : 2048 * (h + 1)
                            ],
                            osb[:],
                        )

    nc.compile()
    return nc


def _get_nc():
    global _CACHED
    if _CACHED is None:
        _CACHED = _build()
    return _CACHED


def make_in_maps(x, wq, bq, wk, bk, wv, bv, wa, ba, coef):
    import ml_dtypes

    x = np.asarray(x, dtype=np.float32)
    xf = np.ascontiguousarray(x.reshape(B, C, N))
    shared = {
        "wqT": np.ascontiguousarray(np.asarray(wq, np.float32).T),
        "wkT": np.ascontiguousarray(np.asarray(wk, np.float32).T),
        "wvT": np.ascontiguousarray(np.asarray(wv, np.float32).T),
        "bq": np.ascontiguousarray(np.asarray(bq, np.float32).reshape(S, 1)),
        "bk": np.ascontiguousarray(np.asarray(bk, np.float32).reshape(S, 1)),
        "bv": np.ascontiguousarray(np.asarray(bv, np.float32).reshape(S, 1)),
        "waT": np.ascontiguousarray(np.asarray(wa, np.float32).T),
        "ba_row": np.ascontiguousarray(np.asarray(ba, np.float32).reshape(1, C)),
        "coefb": np.full((128, 1), np.float32(np.asarray(coef).reshape(-1)[0])),
        "ident128": np.eye(128, dtype=np.float32),
        "ident64": np.eye(S, dtype=ml_dtypes.bfloat16),
        "ones_row": np.ones((1, N), dtype=np.float32),
    }
    return [dict(shared, x=np.ascontiguousarray(xf[b])) for b in range(B)]


def kernel(x, wq, bq, wk, bk, wv, bv, wa, ba, coef, **_unused):
    from concourse.bass_utils import run_bass_kernel_spmd

    nc = _get_nc()
    in_maps = make_in_maps(x, wq, bq, wk, bk, wv, bv, wa, ba, coef)
    res = run_bass_kernel_spmd(nc, in_maps, core_ids=list(range(B)))

    out = np.stack([res.results[b]["out"].reshape(C, H, W) for b in range(B)])
    att = np.stack(
        [res.results[b]["att"].astype(np.float32) for b in range(B)]
    )
    return out, att


# revision 14
# speedup vs baseline: 1.0496x; 1.0446x over previous
"""Trainium2 Bass kernel for the sparse-attention module.

Reference computation (per batch element b):
    q = wq @ x + bq ; k = wk @ x + bk ; v = wv @ x + bv        # [S, N]
    att[i, j] = softmax_j( sum_s k[s, i] * q[s, j] )           # [N, N]
    v2 = v @ att                                               # [S, N]
    out = coef * (wa @ v2 + ba) + x                            # [C, N]
returns (out, att).

Sharding: pure data parallel over batch — B=8 batch elements, one per
NeuronCore. Params are tiny and replicated (pre-transposed on host into
PE-friendly layouts).

Per-core dataflow (C=512, N=4096, S=64):
  phase A: stream x in 512-column tiles, project q/k/v on PE (f32r
           matmuls -> bf16), DMA-xbar-transpose v -> vT (bf16).
           qkv PSUM tiles share one pool with phase B's att quarters so
           the PE instruction stream never pauses long enough for the
           HAM clock gate to re-throttle.
  phase B: per 128-row tile of att: PE k^T q (bf16) into double-buffered
           [128,1024] PSUM quarters, ACT exp -> bf16 SBUF, GpSimd row
           sums, DVE 1/sum scale, DMA att rows out as bf16 (host widens
           to f32), PE v2 accumulation (bf16, col-paired into both PSUM
           partition halves)
  phase C: PE sa = (coef*wa) @ v2_aug (+coef*ba via ones row) + x via
           identity matmul, ACT evacuates PSUM, DMA out
"""

import numpy as np

B, C, H, W = 8, 512, 64, 64
N = H * W          # 4096 tokens
S = C // 8         # 64   small channels
CK = C // 128      # 4    c-chunks (partition blocks of x / out)
NT = N // 128      # 32   n-tiles (att row blocks)
MC = N // 512      # 8    m-chunks (512-wide matmul free dim)

_CACHED = None


def _build():
    import concourse.tile as tile
    from concourse import bacc, mybir

    F32 = mybir.dt.float32
    F32R = mybir.dt.float32r
    BF16 = mybir.dt.bfloat16
    EXP = mybir.ActivationFunctionType.Exp
    COPY = mybir.ActivationFunctionType.Copy
    AX = mybir.AxisListType.X

    nc = bacc.Bacc("TRN2", target_bir_lowering=False, debug=False)

    x_d = nc.dram_tensor("x", [C, N], F32R, kind="ExternalInput")
    wqT_d = nc.dram_tensor("wqT", [C, S], F32R, kind="ExternalInput")
    wkT_d = nc.dram_tensor("wkT", [C, S], F32R, kind="ExternalInput")
    wvT_d = nc.dram_tensor("wvT", [C, S], F32R, kind="ExternalInput")
    bq_d = nc.dram_tensor("bq", [S, 1], F32, kind="ExternalInput")
    bk_d = nc.dram_tensor("bk", [S, 1], F32, kind="ExternalInput")
    bv_d = nc.dram_tensor("bv", [S, 1], F32, kind="ExternalInput")
    waT_d = nc.dram_tensor("waT", [S, C], F32R, kind="ExternalInput")
    ba_row_d = nc.dram_tensor("ba_row", [1, C], F32R, kind="ExternalInput")
    coef_d = nc.dram_tensor("coefb", [128, 1], F32, kind="ExternalInput")
    id128_d = nc.dram_tensor("ident128", [128, 128], F32R, kind="ExternalInput")
    id64_d = nc.dram_tensor("ident64", [S, S], BF16, kind="ExternalInput")
    ones_d = nc.dram_tensor("ones_row", [1, N], F32R, kind="ExternalInput")

    out_d = nc.dram_tensor("out", [C, N], F32, kind="ExternalOutput")
    att_d = nc.dram_tensor("att", [N, N], BF16, kind="ExternalOutput")

    with tile.TileContext(nc) as tc:
        with (
            tc.tile_pool(name="persist", bufs=1) as pp,
            tc.tile_pool(name="attp", bufs=4) as attp,
            tc.tile_pool(name="outp", bufs=2) as outp,
            tc.tile_pool(name="stats", bufs=8) as statp,
        ):
            # fine-grained tiles so consumers wait only on what they need
            xt = {}
            for j in range(MC):
                for kk in range(CK):
                    xt[(j, kk)] = pp.tile([128, 512], F32R, name=f"x_{j}_{kk}")
            qt = [pp.tile([S, 512], BF16, name=f"q_{j}") for j in range(MC)]
            kt = [pp.tile([S, 512], BF16, name=f"k_{j}") for j in range(MC)]
            vt = [pp.tile([S, 512], BF16, name=f"v_{j}") for j in range(MC)]
            vTt = [pp.tile([128, 4, S], BF16, name=f"vT_{j}") for j in range(MC)]
            v2_aug = pp.tile([S + 1, N], F32R)
            v2h_sb = pp.tile([128, N // 2], F32R)
            wqT_sb = pp.tile([128, CK, S], F32R)
            wkT_sb = pp.tile([128, CK, S], F32R)
            wvT_sb = pp.tile([128, CK, S], F32R)
            waT_aug = pp.tile([S + 1, C], F32R)
            bq_sb = pp.tile([S, 1], F32)
            bk_sb = pp.tile([S, 1], F32)
            bv_sb = pp.tile([S, 1], F32)
            coef_sb = pp.tile([128, 1], F32)
            id128_sb = pp.tile([128, 128], F32R)
            id64_sb = pp.tile([S, S], BF16)

            x_re = x_d.ap().rearrange("(kk p) n -> p kk n", p=128)
            for kk in range(CK):
                nc.sync.dma_start(xt[(0, kk)][:], x_re[:, kk, 0:512])
            for w_sb, w_d in ((wqT_sb, wqT_d), (wkT_sb, wkT_d)):
                nc.sync.dma_start(
                    w_sb[:], w_d.ap().rearrange("(kk p) s -> p kk s", p=128)
                )
            nc.sync.dma_start(bq_sb[:], bq_d.ap())
            nc.sync.dma_start(bk_sb[:], bk_d.ap())
            for j in range(1, MC):
                for kk in range(CK):
                    nc.sync.dma_start(
                        xt[(j, kk)][:], x_re[:, kk, 512 * j : 512 * (j + 1)]
                    )
            nc.sync.dma_start(
                wvT_sb[:], wvT_d.ap().rearrange("(kk p) s -> p kk s", p=128)
            )
            nc.sync.dma_start(bv_sb[:], bv_d.ap())
            nc.sync.dma_start(waT_aug[0:S, :], waT_d.ap())
            nc.sync.dma_start(waT_aug[S : S + 1, :], ba_row_d.ap())
            nc.sync.dma_start(coef_sb[:], coef_d.ap())
            nc.sync.dma_start(id128_sb[:], id128_d.ap())
            nc.sync.dma_start(id64_sb[:], id64_d.ap())

            # sa weights scaled by coef on device; row S carries coef*ba
            nc.vector.tensor_scalar_mul(
                waT_aug[:], waT_aug[:], coef_sb[0 : S + 1, :]
            )
            nc.sync.dma_start(v2_aug[S : S + 1, :], ones_d.ap())

            # one shared pool for qkv psum chunks AND att psum quarters:
            # the PE stream flows from projections straight into attention
            # with no pool barrier in between
            with (
                tc.tile_pool(name="psMain", bufs=2, space="PSUM") as psM,
                tc.tile_pool(name="psV", bufs=1, space="PSUM") as psV,
            ):
                # ---------- phase A: q/k projections (v is folded into
                # the first phase-B iterations)
                def project(w_sb, b_sb, dst, j):
                    ps = psM.tile([S, 512], F32, tag="mm", name="ps_prj")
                    for kk in range(CK):
                        nc.tensor.matmul(
                            ps[:],
                            w_sb[:, kk, :],
                            xt[(j, kk)][:],
                            start=(kk == 0),
                            stop=(kk == CK - 1),
                        )
                    nc.vector.tensor_scalar_add(dst[j][:], ps[:], b_sb[:])

                for j in range(MC):
                    project(wqT_sb, bq_sb, qt, j)
                    project(wkT_sb, bk_sb, kt, j)

                def emit_v_chunk(j):
                    project(wvT_sb, bv_sb, vt, j)
                    for i in range(4 * j, 4 * j + 4):
                        pst = psM.tile([128, S], BF16, tag="mm", name="ps_tp")
                        nc.tensor.transpose(
                            pst[:],
                            vt[j][:, 128 * (i % 4) : 128 * (i % 4 + 1)],
                            id64_sb[:],
                        )
                        nc.vector.tensor_copy(vTt[j][:, i % 4, :], pst[:])

                # ---------- phase B: attention rows + v2 accumulation
                v2ps = psV.tile([128, 4 * 512], F32)

                def emit_v2(i, asb_i, jlist):
                    # interleave the two psum col-groups so the pairs run
                    # concurrently on the PE quadrants
                    for j in jlist:
                        rb = 64 * (j // 4)
                        nc.tensor.matmul(
                            v2ps[rb : rb + 64, 512 * (j % 4) : 512 * (j % 4) + 512],
                            vTt[i // 4][:, i % 4, :],
                            asb_i[:, 512 * j : 512 * (j + 1)],
                            start=(i == 0),
                            stop=(i == NT - 1),
                            tile_position=(0, rb),
                            skip_group_check=True,
                        )

                prev = None  # (i, asb) whose v2 matmuls are still pending
                for j in range(MC):
                    emit_v_chunk(j)
                for i in range(NT):
                    asb = attp.tile([128, N], BF16)
                    st = statp.tile([128, 8], F32)
                    kt_i = kt[i // 4]
                    ko = 128 * (i % 4)
                    for qq in range(4):
                        aps = psM.tile([128, 1024], F32, tag="mm")
                        for j in range(2):
                            nc.tensor.matmul(
                                aps[:, 512 * j : 512 * (j + 1)],
                                kt_i[:, ko : ko + 128],
                                qt[2 * qq + j][:],
                                start=True,
                                stop=True,
                            )
                        nc.scalar.activation(
                            asb[:, 1024 * qq : 1024 * (qq + 1)],
                            aps[:],
                            EXP,
                            accum_out=st[:, qq : qq + 1],
                        )
                        if qq == 1 and prev is not None:
                            emit_v2(prev[0], prev[1], [0, 4, 1, 5])
                        if qq == 3 and prev is not None:
                            emit_v2(prev[0], prev[1], [2, 6])
                    nc.vector.reduce_sum(st[:, 4:5], st[:, 0:4], axis=AX)
                    nc.vector.reciprocal(st[:, 5:6], st[:, 4:5])
                    nc.vector.tensor_scalar_mul(asb[:], asb[:], st[:, 5:6])
                    nc.sync.dma_start(att_d.ap()[128 * i : 128 * (i + 1), :], asb[:])
                    if prev is not None:
                        emit_v2(prev[0], prev[1], [3, 7])
                    prev = (i, asb)
                emit_v2(prev[0], prev[1], [0, 4, 1, 5, 2, 6, 3, 7])

                # v2ps rows 0-63 hold m-chunks 0-3; rows 64-127 hold 4-7.
                # Everything must land on partitions 0-64 for the sa matmuls:
                # lower half straight via DVE, upper half DVE->SBUF then a
                # cross-partition SBUF->SBUF DMA.
                nc.vector.tensor_copy(v2_aug[0:S, 0 : N // 2], v2ps[0:64, :])
                nc.vector.tensor_copy(v2h_sb[64:128, :], v2ps[64:128, :])
                nc.sync.dma_start(v2_aug[0:S, N // 2 : N], v2h_sb[64:128, :])

            # ---------- phase C: out = coef*(wa@v2 + ba) + x, all on PE
            with tc.tile_pool(name="psD", bufs=2, space="PSUM") as psD:
                for kk in range(CK):
                    for h in range(2):
                        sps = psD.tile([128, 2048], F32)
                        for j in range(4):
                            jj = 4 * h + j
                            nc.tensor.matmul(
                                sps[:, 512 * j : 512 * (j + 1)],
                                waT_aug[:, 128 * kk : 128 * (kk + 1)],
                                v2_aug[:, 512 * jj : 512 * (jj + 1)],
                                start=True,
                                stop=False,
                                skip_group_check=True,
                            )
                            nc.tensor.matmul(
                                sps[:, 512 * j : 512 * (j + 1)],
                                id128_sb[:],
                                xt[(jj, kk)][:],
                                start=False,
                                stop=True,
                                skip_group_check=True,
                            )
                        osb = outp.tile([128, 2048], F32)
                        nc.scalar.activation(osb[:], sps[:], COPY)
                        nc.sync.dma_start(
                            out_d.ap()[
                                128 * kk : 128 * (kk + 1), 2048 * h : 2048 * (h + 1)
                            ],
                            osb[:],
                        )

    nc.compile()
    return nc


def _get_nc():
    global _CACHED
    if _CACHED is None:
        _CACHED = _build()
    return _CACHED


def make_in_maps(x, wq, bq, wk, bk, wv, bv, wa, ba, coef):
    import ml_dtypes

    x = np.asarray(x, dtype=np.float32)
    xf = np.ascontiguousarray(x.reshape(B, C, N))
    shared = {
        "wqT": np.ascontiguousarray(np.asarray(wq, np.float32).T),
        "wkT": np.ascontiguousarray(np.asarray(wk, np.float32).T),
        "wvT": np.ascontiguousarray(np.asarray(wv, np.float32).T),
        "bq": np.ascontiguousarray(np.asarray(bq, np.float32).reshape(S, 1)),
        "bk": np.ascontiguousarray(np.asarray(bk, np.float32).reshape(S, 1)),
        "bv": np.ascontiguousarray(np.asarray(bv, np.float32).reshape(S, 1)),
        "waT": np.ascontiguousarray(np.asarray(wa, np.float32).T),
        "ba_row": np.ascontiguousarray(np.asarray(ba, np.float32).reshape(1, C)),
        "coefb": np.full((128, 1), np.float32(np.asarray(coef).reshape(-1)[0])),
        "ident128": np.eye(128, dtype=np.float32),
        "ident64": np.eye(S, dtype=ml_dtypes.bfloat16),
        "ones_row": np.ones((1, N), dtype=np.float32),
    }
    return [dict(shared, x=np.ascontiguousarray(xf[b])) for b in range(B)]


def kernel(x, wq, bq, wk, bk, wv, bv, wa, ba, coef, **_unused):
    from concourse.bass_utils import run_bass_kernel_spmd

    nc = _get_nc()
    in_maps = make_in_maps(x, wq, bq, wk, bk, wv, bv, wa, ba, coef)
    res = run_bass_kernel_spmd(nc, in_maps, core_ids=list(range(B)))

    out = np.stack([res.results[b]["out"].reshape(C, H, W) for b in range(B)])
    att = np.stack(
        [res.results[b]["att"].astype(np.float32) for b in range(B)]
    )
    return out, att


# revision 17
# speedup vs baseline: 1.0748x; 1.0240x over previous
"""Trainium2 Bass kernel for the sparse-attention module.

Reference computation (per batch element b):
    q = wq @ x + bq ; k = wk @ x + bk ; v = wv @ x + bv        # [S, N]
    att[i, j] = softmax_j( sum_s k[s, i] * q[s, j] )           # [N, N]
    v2 = v @ att                                               # [S, N]
    out = coef * (wa @ v2 + ba) + x                            # [C, N]
returns (out, att).

Sharding: pure data parallel over batch — B=8 batch elements, one per
NeuronCore. Params are tiny and replicated (pre-transposed on host into
PE-friendly layouts).

Per-core dataflow (C=512, N=4096, S=64):
  phase A: stream x in 512-column tiles, project q/k/v on PE (f32r
           matmuls -> bf16), DMA-xbar-transpose v -> vT (bf16).
           qkv PSUM tiles share one pool with phase B's att quarters so
           the PE instruction stream never pauses long enough for the
           HAM clock gate to re-throttle.
  phase B: per 128-row tile of att: PE k^T q (bf16) into double-buffered
           [128,1024] PSUM quarters, ACT exp -> bf16 SBUF, GpSimd row
           sums, DVE 1/sum scale, DMA att rows out as bf16 (host widens
           to f32), PE v2 accumulation (bf16, col-paired into both PSUM
           partition halves)
  phase C: PE sa = (coef*wa) @ v2_aug (+coef*ba via ones row) + x via
           identity matmul, ACT evacuates PSUM, DMA out
"""

import numpy as np

B, C, H, W = 8, 512, 64, 64
N = H * W          # 4096 tokens
S = C // 8         # 64   small channels
CK = C // 128      # 4    c-chunks (partition blocks of x / out)
NT = N // 128      # 32   n-tiles (att row blocks)
MC = N // 512      # 8    m-chunks (512-wide matmul free dim)

_CACHED = None


def _build():
    import concourse.tile as tile
    from concourse import bacc, mybir

    F32 = mybir.dt.float32
    F32R = mybir.dt.float32r
    BF16 = mybir.dt.bfloat16
    EXP = mybir.ActivationFunctionType.Exp
    COPY = mybir.ActivationFunctionType.Copy
    AX = mybir.AxisListType.X

    nc = bacc.Bacc("TRN2", target_bir_lowering=False, debug=False)

    x_d = nc.dram_tensor("x", [C, N], F32R, kind="ExternalInput")
    wqT_d = nc.dram_tensor("wqT", [C, S], F32R, kind="ExternalInput")
    wkT_d = nc.dram_tensor("wkT", [C, S], F32R, kind="ExternalInput")
    wvT_d = nc.dram_tensor("wvT", [C, S], F32R, kind="ExternalInput")
    bq_d = nc.dram_tensor("bq", [S, 1], F32, kind="ExternalInput")
    bk_d = nc.dram_tensor("bk", [S, 1], F32, kind="ExternalInput")
    bv_d = nc.dram_tensor("bv", [S, 1], F32, kind="ExternalInput")
    waT_d = nc.dram_tensor("waT", [S, C], F32R, kind="ExternalInput")
    ba_row_d = nc.dram_tensor("ba_row", [1, C], F32R, kind="ExternalInput")
    coef_d = nc.dram_tensor("coefb", [128, 1], F32, kind="ExternalInput")
    id128_d = nc.dram_tensor("ident128", [128, 128], F32R, kind="ExternalInput")
    id64_d = nc.dram_tensor("ident64", [S, S], BF16, kind="ExternalInput")
    ones_d = nc.dram_tensor("ones_row", [1, N], F32R, kind="ExternalInput")

    out_d = nc.dram_tensor("out", [C, N], F32, kind="ExternalOutput")
    att_d = nc.dram_tensor("att", [N, N], BF16, kind="ExternalOutput")

    with tile.TileContext(nc) as tc:
        with (
            tc.tile_pool(name="persist", bufs=1) as pp,
            tc.tile_pool(name="attp", bufs=3) as attp,
            tc.tile_pool(name="attop", bufs=2) as attop,
            tc.tile_pool(name="outp", bufs=2) as outp,
            tc.tile_pool(name="stats", bufs=8) as statp,
        ):
            # fine-grained tiles so consumers wait only on what they need
            xt = {}
            for j in range(MC):
                for kk in range(CK):
                    xt[(j, kk)] = pp.tile([128, 512], F32R, name=f"x_{j}_{kk}")
            qt = [pp.tile([S, 512], BF16, name=f"q_{j}") for j in range(MC)]
            kt = [pp.tile([S, 512], BF16, name=f"k_{j}") for j in range(MC)]
            vt = [pp.tile([S, 512], BF16, name=f"v_{j}") for j in range(MC)]
            vTt = [pp.tile([128, 4, S], BF16, name=f"vT_{j}") for j in range(MC)]
            v2_aug = pp.tile([S + 1, N], F32R)
            v2h_sb = pp.tile([128, N // 2], F32R)
            wqT_sb = pp.tile([128, CK, S], F32R)
            wkT_sb = pp.tile([128, CK, S], F32R)
            wvT_sb = pp.tile([128, CK, S], F32R)
            waT_aug = pp.tile([S + 1, C], F32R)
            bq_sb = pp.tile([S, 1], F32)
            bk_sb = pp.tile([S, 1], F32)
            bv_sb = pp.tile([S, 1], F32)
            coef_sb = pp.tile([128, 1], F32)
            id128_sb = pp.tile([128, 128], F32R)
            id64_sb = pp.tile([S, S], BF16)

            x_re = x_d.ap().rearrange("(kk p) n -> p kk n", p=128)
            for kk in range(CK):
                nc.sync.dma_start(xt[(0, kk)][:], x_re[:, kk, 0:512])
            for w_sb, w_d in ((wqT_sb, wqT_d), (wkT_sb, wkT_d)):
                nc.sync.dma_start(
                    w_sb[:], w_d.ap().rearrange("(kk p) s -> p kk s", p=128)
                )
            nc.sync.dma_start(bq_sb[:], bq_d.ap())
            nc.sync.dma_start(bk_sb[:], bk_d.ap())
            for j in range(1, MC):
                for kk in range(CK):
                    nc.sync.dma_start(
                        xt[(j, kk)][:], x_re[:, kk, 512 * j : 512 * (j + 1)]
                    )
            nc.sync.dma_start(
                wvT_sb[:], wvT_d.ap().rearrange("(kk p) s -> p kk s", p=128)
            )
            nc.sync.dma_start(bv_sb[:], bv_d.ap())
            nc.sync.dma_start(waT_aug[0:S, :], waT_d.ap())
            nc.sync.dma_start(waT_aug[S : S + 1, :], ba_row_d.ap())
            nc.sync.dma_start(coef_sb[:], coef_d.ap())
            nc.sync.dma_start(id128_sb[:], id128_d.ap())
            nc.sync.dma_start(id64_sb[:], id64_d.ap())

            # sa weights scaled by coef on device; row S carries coef*ba
            nc.vector.tensor_scalar_mul(
                waT_aug[:], waT_aug[:], coef_sb[0 : S + 1, :]
            )
            nc.sync.dma_start(v2_aug[S : S + 1, :], ones_d.ap())

            # one shared pool for qkv psum chunks AND att psum quarters:
            # the PE stream flows from projections straight into attention
            # with no pool barrier in between
            with (
                tc.tile_pool(name="psMain", bufs=2, space="PSUM") as psM,
                tc.tile_pool(name="psV", bufs=1, space="PSUM") as psV,
            ):
                # ---------- phase A: q/k projections (v is folded into
                # the first phase-B iterations)
                def project(w_sb, b_sb, dst, j):
                    ps = psM.tile([S, 512], F32, tag="mm", name="ps_prj")
                    for kk in range(CK):
                        nc.tensor.matmul(
                            ps[:],
                            w_sb[:, kk, :],
                            xt[(j, kk)][:],
                            start=(kk == 0),
                            stop=(kk == CK - 1),
                        )
                    nc.vector.tensor_scalar_add(dst[j][:], ps[:], b_sb[:])

                for j in range(MC):
                    project(wqT_sb, bq_sb, qt, j)
                    project(wkT_sb, bk_sb, kt, j)

                def emit_v_chunk(j):
                    project(wvT_sb, bv_sb, vt, j)
                    pst = psM.tile([128, 4, S], BF16, tag="mm", name="ps_tp")
                    for t in range(4):
                        nc.tensor.transpose(
                            pst[:, t, :],
                            vt[j][:, 128 * t : 128 * (t + 1)],
                            id64_sb[:],
                        )
                    nc.vector.tensor_copy(vTt[j][:], pst[:])

                # ---------- phase B: attention rows + v2 accumulation
                v2ps = psV.tile([128, 4 * 512], F32)

                def emit_v2(i, asb_i, vTs_i, jlist):
                    # interleave the two psum col-groups so the pairs run
                    # concurrently on the PE quadrants; vTs already carries
                    # the 1/rowsum normalization so the unscaled exp works
                    # as the moving operand
                    for j in jlist:
                        rb = 64 * (j // 4)
                        nc.tensor.matmul(
                            v2ps[rb : rb + 64, 512 * (j % 4) : 512 * (j % 4) + 512],
                            vTs_i[:],
                            asb_i[:, 512 * j : 512 * (j + 1)],
                            start=(i == 0),
                            stop=(i == NT - 1),
                            tile_position=(0, rb),
                            skip_group_check=True,
                        )

                prev = None  # (i, asb) whose v2 matmuls are still pending
                for j in range(MC):
                    emit_v_chunk(j)
                for i in range(NT):
                    asb = attp.tile([128, N], BF16)
                    st = statp.tile([128, 8], F32)
                    kt_i = kt[i // 4]
                    ko = 128 * (i % 4)
                    for qq in range(4):
                        aps = psM.tile([128, 1024], F32, tag="mm")
                        for j in range(2):
                            nc.tensor.matmul(
                                aps[:, 512 * j : 512 * (j + 1)],
                                kt_i[:, ko : ko + 128],
                                qt[2 * qq + j][:],
                                start=True,
                                stop=True,
                            )
                        nc.scalar.activation(
                            asb[:, 1024 * qq : 1024 * (qq + 1)],
                            aps[:],
                            EXP,
                            accum_out=st[:, qq : qq + 1],
                        )
                        if qq == 1 and prev is not None:
                            emit_v2(prev[0], prev[1], prev[2], [0, 4, 1, 5])
                        if qq == 3 and prev is not None:
                            emit_v2(prev[0], prev[1], prev[2], [2, 6])
                    nc.vector.reduce_sum(st[:, 4:5], st[:, 0:4], axis=AX)
                    nc.vector.reciprocal(st[:, 5:6], st[:, 4:5])
                    vTs = statp.tile([128, S], BF16, name="vT_scaled")
                    nc.vector.tensor_scalar_mul(
                        vTs[:], vTt[i // 4][:, i % 4, :], st[:, 5:6]
                    )
                    aob = attop.tile([128, N], BF16)
                    nc.vector.tensor_scalar_mul(aob[:], asb[:], st[:, 5:6])
                    nc.sync.dma_start(att_d.ap()[128 * i : 128 * (i + 1), :], aob[:])
                    if prev is not None:
                        emit_v2(prev[0], prev[1], prev[2], [3, 7])
                    prev = (i, asb, vTs)
                emit_v2(prev[0], prev[1], prev[2], [0, 4, 1, 5, 2, 6, 3, 7])

                # v2ps rows 0-63 hold m-chunks 0-3; rows 64-127 hold 4-7.
                # Everything must land on partitions 0-64 for the sa matmuls:
                # lower half straight via DVE, upper half DVE->SBUF then a
                # cross-partition SBUF->SBUF DMA.
                nc.vector.tensor_copy(v2_aug[0:S, 0 : N // 2], v2ps[0:64, :])
                nc.vector.tensor_copy(v2h_sb[64:128, :], v2ps[64:128, :])
                nc.sync.dma_start(v2_aug[0:S, N // 2 : N], v2h_sb[64:128, :])

            # ---------- phase C: out = coef*(wa@v2 + ba) + x, all on PE
            with tc.tile_pool(name="psD", bufs=2, space="PSUM") as psD:
                for kk in range(CK):
                    for h in range(2):
                        sps = psD.tile([128, 2048], F32)
                        for j in range(4):
                            jj = 4 * h + j
                            nc.tensor.matmul(
                                sps[:, 512 * j : 512 * (j + 1)],
                                waT_aug[:, 128 * kk : 128 * (kk + 1)],
                                v2_aug[:, 512 * jj : 512 * (jj + 1)],
                                start=True,
                                stop=False,
                                skip_group_check=True,
                            )
                            nc.tensor.matmul(
                                sps[:, 512 * j : 512 * (j + 1)],
                                id128_sb[:],
                                xt[(jj, kk)][:],
                                start=False,
                                stop=True,
                                skip_group_check=True,
                            )
                        osb = outp.tile([128, 2048], F32)
                        nc.scalar.activation(osb[:], sps[:], COPY)
                        nc.sync.dma_start(
                            out_d.ap()[
                                128 * kk : 128 * (kk + 1), 2048 * h : 2048 * (h + 1)
                            ],
                            osb[:],
                        )

    nc.compile()
    return nc


def _get_nc():
    global _CACHED
    if _CACHED is None:
        _CACHED = _build()
    return _CACHED


def make_in_maps(x, wq, bq, wk, bk, wv, bv, wa, ba, coef):
    import ml_dtypes

    x = np.asarray(x, dtype=np.float32)
    xf = np.ascontiguousarray(x.reshape(B, C, N))
    shared = {
        "wqT": np.ascontiguousarray(np.asarray(wq, np.float32).T),
        "wkT": np.ascontiguousarray(np.asarray(wk, np.float32).T),
        "wvT": np.ascontiguousarray(np.asarray(wv, np.float32).T),
        "bq": np.ascontiguousarray(np.asarray(bq, np.float32).reshape(S, 1)),
        "bk": np.ascontiguousarray(np.asarray(bk, np.float32).reshape(S, 1)),
        "bv": np.ascontiguousarray(np.asarray(bv, np.float32).reshape(S, 1)),
        "waT": np.ascontiguousarray(np.asarray(wa, np.float32).T),
        "ba_row": np.ascontiguousarray(np.asarray(ba, np.float32).reshape(1, C)),
        "coefb": np.full((128, 1), np.float32(np.asarray(coef).reshape(-1)[0])),
        "ident128": np.eye(128, dtype=np.float32),
        "ident64": np.eye(S, dtype=ml_dtypes.bfloat16),
        "ones_row": np.ones((1, N), dtype=np.float32),
    }
    return [dict(shared, x=np.ascontiguousarray(xf[b])) for b in range(B)]


def kernel(x, wq, bq, wk, bk, wv, bv, wa, ba, coef, **_unused):
    from concourse.bass_utils import run_bass_kernel_spmd

    nc = _get_nc()
    in_maps = make_in_maps(x, wq, bq, wk, bk, wv, bv, wa, ba, coef)
    res = run_bass_kernel_spmd(nc, in_maps, core_ids=list(range(B)))

    out = np.stack([res.results[b]["out"].reshape(C, H, W) for b in range(B)])
    att = np.stack(
        [res.results[b]["att"].astype(np.float32) for b in range(B)]
    )
    return out, att


# revision 18
# speedup vs baseline: 1.0952x; 1.0190x over previous
"""Trainium2 Bass kernel for the sparse-attention module.

Reference computation (per batch element b):
    q = wq @ x + bq ; k = wk @ x + bk ; v = wv @ x + bv        # [S, N]
    att[i, j] = softmax_j( sum_s k[s, i] * q[s, j] )           # [N, N]
    v2 = v @ att                                               # [S, N]
    out = coef * (wa @ v2 + ba) + x                            # [C, N]
returns (out, att).

Sharding: pure data parallel over batch — B=8 batch elements, one per
NeuronCore. Params are tiny and replicated (pre-transposed on host into
PE-friendly layouts).

Per-core dataflow (C=512, N=4096, S=64):
  phase A: stream x in 512-column tiles, project q/k/v on PE (f32r
           matmuls -> bf16), DMA-xbar-transpose v -> vT (bf16).
           qkv PSUM tiles share one pool with phase B's att quarters so
           the PE instruction stream never pauses long enough for the
           HAM clock gate to re-throttle.
  phase B: per 128-row tile of att: PE k^T q (bf16) into double-buffered
           [128,1024] PSUM quarters, ACT exp -> bf16 SBUF, GpSimd row
           sums, DVE 1/sum scale, DMA att rows out as bf16 (host widens
           to f32), PE v2 accumulation (bf16, col-paired into both PSUM
           partition halves)
  phase C: PE sa = (coef*wa) @ v2_aug (+coef*ba via ones row) + x via
           identity matmul, ACT evacuates PSUM, DMA out
"""

import numpy as np

B, C, H, W = 8, 512, 64, 64
N = H * W          # 4096 tokens
S = C // 8         # 64   small channels
CK = C // 128      # 4    c-chunks (partition blocks of x / out)
NT = N // 128      # 32   n-tiles (att row blocks)
MC = N // 512      # 8    m-chunks (512-wide matmul free dim)

_CACHED = None


def _build():
    import concourse.tile as tile
    from concourse import bacc, mybir

    F32 = mybir.dt.float32
    F32R = mybir.dt.float32r
    BF16 = mybir.dt.bfloat16
    EXP = mybir.ActivationFunctionType.Exp
    COPY = mybir.ActivationFunctionType.Copy
    MULT = mybir.AluOpType.mult
    ADD = mybir.AluOpType.add
    AX = mybir.AxisListType.X

    nc = bacc.Bacc("TRN2", target_bir_lowering=False, debug=False)

    x_d = nc.dram_tensor("x", [C, N], F32R, kind="ExternalInput")
    wqT_d = nc.dram_tensor("wqT", [C, S], F32R, kind="ExternalInput")
    wkT_d = nc.dram_tensor("wkT", [C, S], F32R, kind="ExternalInput")
    wvT_d = nc.dram_tensor("wvT", [C, S], F32R, kind="ExternalInput")
    bq_d = nc.dram_tensor("bq", [S, 1], F32, kind="ExternalInput")
    bk_d = nc.dram_tensor("bk", [S, 1], F32, kind="ExternalInput")
    bv_d = nc.dram_tensor("bv", [S, 1], F32, kind="ExternalInput")
    waT_d = nc.dram_tensor("waT", [S, C], F32R, kind="ExternalInput")
    ba_row_d = nc.dram_tensor("ba_row", [1, C], F32R, kind="ExternalInput")
    coef_d = nc.dram_tensor("coefb", [128, 1], F32, kind="ExternalInput")
    id128_d = nc.dram_tensor("ident128", [128, 128], F32R, kind="ExternalInput")
    id64_d = nc.dram_tensor("ident64", [S, S], BF16, kind="ExternalInput")
    ones_d = nc.dram_tensor("ones_row", [1, N], F32R, kind="ExternalInput")

    out_d = nc.dram_tensor("out", [C, N], F32, kind="ExternalOutput")
    att_d = nc.dram_tensor("att", [N, N], BF16, kind="ExternalOutput")

    with tile.TileContext(nc) as tc:
        with (
            tc.tile_pool(name="persist", bufs=1) as pp,
            tc.tile_pool(name="attp", bufs=3) as attp,
            tc.tile_pool(name="attop", bufs=2) as attop,
            tc.tile_pool(name="outp", bufs=2) as outp,
            tc.tile_pool(name="stats", bufs=8) as statp,
        ):
            # fine-grained tiles so consumers wait only on what they need
            xt = {}
            for j in range(MC):
                for kk in range(CK):
                    xt[(j, kk)] = pp.tile([128, 512], F32R, name=f"x_{j}_{kk}")
            qt = [pp.tile([S, 512], BF16, name=f"q_{j}") for j in range(MC)]
            kt = [pp.tile([S, 512], BF16, name=f"k_{j}") for j in range(MC)]
            vt = [pp.tile([S, 512], BF16, name=f"v_{j}") for j in range(MC)]
            vTt = [pp.tile([128, 4, S], BF16, name=f"vT_{j}") for j in range(MC)]
            v2_aug = pp.tile([S + 1, N], F32R)
            v2h_sb = pp.tile([128, N // 2], F32R)
            wqT_sb = pp.tile([128, CK, S], F32R)
            wkT_sb = pp.tile([128, CK, S], F32R)
            wvT_sb = pp.tile([128, CK, S], F32R)
            waT_aug = pp.tile([S + 1, C], F32R)
            bq_sb = pp.tile([S, 1], F32)
            bk_sb = pp.tile([S, 1], F32)
            bv_sb = pp.tile([S, 1], F32)
            coef_sb = pp.tile([128, 1], F32)
            id128_sb = pp.tile([128, 128], F32R)
            id64_sb = pp.tile([S, S], BF16)

            x_re = x_d.ap().rearrange("(kk p) n -> p kk n", p=128)
            for kk in range(CK):
                nc.sync.dma_start(xt[(0, kk)][:], x_re[:, kk, 0:512])
            for w_sb, w_d in ((wqT_sb, wqT_d), (wkT_sb, wkT_d)):
                nc.sync.dma_start(
                    w_sb[:], w_d.ap().rearrange("(kk p) s -> p kk s", p=128)
                )
            nc.sync.dma_start(bq_sb[:], bq_d.ap())
            nc.sync.dma_start(bk_sb[:], bk_d.ap())
            for j in range(1, MC):
                for kk in range(CK):
                    nc.sync.dma_start(
                        xt[(j, kk)][:], x_re[:, kk, 512 * j : 512 * (j + 1)]
                    )
            nc.sync.dma_start(
                wvT_sb[:], wvT_d.ap().rearrange("(kk p) s -> p kk s", p=128)
            )
            nc.sync.dma_start(bv_sb[:], bv_d.ap())
            nc.sync.dma_start(waT_aug[0:S, :], waT_d.ap())
            nc.sync.dma_start(waT_aug[S : S + 1, :], ba_row_d.ap())
            nc.sync.dma_start(coef_sb[:], coef_d.ap())
            nc.sync.dma_start(id128_sb[:], id128_d.ap())
            nc.sync.dma_start(id64_sb[:], id64_d.ap())

            # sa weights scaled by coef on device; row S carries coef*ba
            nc.vector.tensor_scalar_mul(
                waT_aug[:], waT_aug[:], coef_sb[0 : S + 1, :]
            )
            nc.sync.dma_start(v2_aug[S : S + 1, :], ones_d.ap())

            # one shared pool for qkv psum chunks AND att psum quarters:
            # the PE stream flows from projections straight into attention
            # with no pool barrier in between
            with (
                tc.tile_pool(name="psMain", bufs=2, space="PSUM") as psM,
                tc.tile_pool(name="psV", bufs=1, space="PSUM") as psV,
            ):
                # ---------- phase A: q/k projections (v is folded into
                # the first phase-B iterations)
                def project(w_sb, b_sb, dst, j):
                    ps = psM.tile([S, 512], F32, tag="mm", name="ps_prj")
                    for kk in range(CK):
                        nc.tensor.matmul(
                            ps[:],
                            w_sb[:, kk, :],
                            xt[(j, kk)][:],
                            start=(kk == 0),
                            stop=(kk == CK - 1),
                        )
                    nc.vector.tensor_scalar_add(dst[j][:], ps[:], b_sb[:])

                for j in range(MC):
                    project(wqT_sb, bq_sb, qt, j)
                    project(wkT_sb, bk_sb, kt, j)

                def emit_v_chunk(j):
                    pst = psM.tile([128, 4, S], BF16, tag="mm", name="ps_tp")
                    for t in range(4):
                        nc.tensor.transpose(
                            pst[:, t, :],
                            vt[j][:, 128 * t : 128 * (t + 1)],
                            id64_sb[:],
                        )
                    nc.vector.tensor_copy(vTt[j][:], pst[:])

                # ---------- phase B: attention rows + v2 accumulation
                v2ps = psV.tile([128, 4 * 512], F32)

                def emit_v2(i, asb_i, vTs_i, jlist):
                    # interleave the two psum col-groups so the pairs run
                    # concurrently on the PE quadrants; vTs already carries
                    # the 1/rowsum normalization so the unscaled exp works
                    # as the moving operand
                    for j in jlist:
                        rb = 64 * (j // 4)
                        nc.tensor.matmul(
                            v2ps[rb : rb + 64, 512 * (j % 4) : 512 * (j % 4) + 512],
                            vTs_i[:],
                            asb_i[:, 512 * j : 512 * (j + 1)],
                            start=(i == 0),
                            stop=(i == NT - 1),
                            tile_position=(0, rb),
                            skip_group_check=True,
                        )

                prev = None  # (i, asb) whose v2 matmuls are still pending
                for j in range(MC):
                    project(wvT_sb, bv_sb, vt, j)
                for j in range(MC):
                    emit_v_chunk(j)
                for i in range(NT):
                    asb = attp.tile([128, N], BF16)
                    st = statp.tile([128, 8], F32)
                    kt_i = kt[i // 4]
                    ko = 128 * (i % 4)
                    for qq in range(4):
                        aps = psM.tile([128, 1024], F32, tag="mm")
                        for j in range(2):
                            nc.tensor.matmul(
                                aps[:, 512 * j : 512 * (j + 1)],
                                kt_i[:, ko : ko + 128],
                                qt[2 * qq + j][:],
                                start=True,
                                stop=True,
                            )
                        nc.scalar.activation(
                            asb[:, 1024 * qq : 1024 * (qq + 1)],
                            aps[:],
                            EXP,
                            accum_out=st[:, qq : qq + 1],
                        )
                        if qq == 1 and prev is not None:
                            emit_v2(
                                prev[0], prev[1], prev[2],
                                [0, 4, 1, 5, 2, 6, 3, 7],
                            )
                    nc.vector.reduce_sum(st[:, 4:5], st[:, 0:4], axis=AX)
                    nc.vector.reciprocal(st[:, 5:6], st[:, 4:5])
                    vTs = statp.tile([128, S], BF16, name="vT_scaled")
                    nc.vector.tensor_scalar_mul(
                        vTs[:], vTt[i // 4][:, i % 4, :], st[:, 5:6]
                    )
                    aob = attop.tile([128, N], BF16)
                    nc.vector.tensor_scalar_mul(aob[:], asb[:], st[:, 5:6])
                    nc.sync.dma_start(att_d.ap()[128 * i : 128 * (i + 1), :], aob[:])
                    prev = (i, asb, vTs)
                emit_v2(prev[0], prev[1], prev[2], [0, 4, 1, 5, 2, 6, 3, 7])

                # v2ps rows 0-63 hold m-chunks 0-3; rows 64-127 hold 4-7.
                # Everything must land on partitions 0-64 for the sa matmuls:
                # lower half straight via DVE, upper half DVE->SBUF then a
                # cross-partition SBUF->SBUF DMA.
                nc.vector.tensor_copy(v2_aug[0:S, 0 : N // 2], v2ps[0:64, :])
                nc.vector.tensor_copy(v2h_sb[64:128, :], v2ps[64:128, :])
                nc.sync.dma_start(v2_aug[0:S, N // 2 : N], v2h_sb[64:128, :])

            # ---------- phase C: out = coef*(wa@v2 + ba) + x, all on PE
            with tc.tile_pool(name="psD", bufs=2, space="PSUM") as psD:
                for kk in range(CK):
                    for h in range(2):
                        sps = psD.tile([128, 2048], F32)
                        for j in range(4):
                            jj = 4 * h + j
                            nc.tensor.matmul(
                                sps[:, 512 * j : 512 * (j + 1)],
                                waT_aug[:, 128 * kk : 128 * (kk + 1)],
                                v2_aug[:, 512 * jj : 512 * (jj + 1)],
                                start=True,
                                stop=True,
                            )
                        osb = outp.tile([128, 2048], F32)
                        for j in range(4):
                            jj = 4 * h + j
                            # out = sps (= coef*(wa@v2+ba)) + x
                            nc.vector.scalar_tensor_tensor(
                                osb[:, 512 * j : 512 * (j + 1)],
                                sps[:, 512 * j : 512 * (j + 1)],
                                1.0,
                                xt[(jj, kk)][:].bitcast(F32),
                                op0=MULT,
                                op1=ADD,
                            )
                        nc.sync.dma_start(
                            out_d.ap()[
                                128 * kk : 128 * (kk + 1), 2048 * h : 2048 * (h + 1)
                            ],
                            osb[:],
                        )

    nc.compile()
    return nc


def _get_nc():
    global _CACHED
    if _CACHED is None:
        _CACHED = _build()
    return _CACHED


def make_in_maps(x, wq, bq, wk, bk, wv, bv, wa, ba, coef):
    import ml_dtypes

    x = np.asarray(x, dtype=np.float32)
    xf = np.ascontiguousarray(x.reshape(B, C, N))
    shared = {
        "wqT": np.ascontiguousarray(np.asarray(wq, np.float32).T),
        "wkT": np.ascontiguousarray(np.asarray(wk, np.float32).T),
        "wvT": np.ascontiguousarray(np.asarray(wv, np.float32).T),
        "bq": np.ascontiguousarray(np.asarray(bq, np.float32).reshape(S, 1)),
        "bk": np.ascontiguousarray(np.asarray(bk, np.float32).reshape(S, 1)),
        "bv": np.ascontiguousarray(np.asarray(bv, np.float32).reshape(S, 1)),
        "waT": np.ascontiguousarray(np.asarray(wa, np.float32).T),
        "ba_row": np.ascontiguousarray(np.asarray(ba, np.float32).reshape(1, C)),
        "coefb": np.full((128, 1), np.float32(np.asarray(coef).reshape(-1)[0])),
        "ident128": np.eye(128, dtype=np.float32),
        "ident64": np.eye(S, dtype=ml_dtypes.bfloat16),
        "ones_row": np.ones((1, N), dtype=np.float32),
    }
    return [dict(shared, x=np.ascontiguousarray(xf[b])) for b in range(B)]


def kernel(x, wq, bq, wk, bk, wv, bv, wa, ba, coef, **_unused):
    from concourse.bass_utils import run_bass_kernel_spmd

    nc = _get_nc()
    in_maps = make_in_maps(x, wq, bq, wk, bk, wv, bv, wa, ba, coef)
    res = run_bass_kernel_spmd(nc, in_maps, core_ids=list(range(B)))

    out = np.stack([res.results[b]["out"].reshape(C, H, W) for b in range(B)])
    att = np.stack(
        [res.results[b]["att"].astype(np.float32) for b in range(B)]
    )
    return out, att


# revision 19
# speedup vs baseline: 1.1309x; 1.0326x over previous
"""Trainium2 Bass kernel for the sparse-attention module.

Reference computation (per batch element b):
    q = wq @ x + bq ; k = wk @ x + bk ; v = wv @ x + bv        # [S, N]
    att[i, j] = softmax_j( sum_s k[s, i] * q[s, j] )           # [N, N]
    v2 = v @ att                                               # [S, N]
    out = coef * (wa @ v2 + ba) + x                            # [C, N]
returns (out, att).

Sharding: pure data parallel over batch — B=8 batch elements, one per
NeuronCore. Params are tiny and replicated (pre-transposed on host into
PE-friendly layouts).

Per-core dataflow (C=512, N=4096, S=64):
  phase A: stream x in 512-column tiles, project q/k/v on PE (f32r
           matmuls -> bf16), DMA-xbar-transpose v -> vT (bf16).
           qkv PSUM tiles share one pool with phase B's att quarters so
           the PE instruction stream never pauses long enough for the
           HAM clock gate to re-throttle.
  phase B: per 128-row tile of att: PE k^T q (bf16) into double-buffered
           [128,1024] PSUM quarters, ACT exp -> bf16 SBUF, GpSimd row
           sums, DVE 1/sum scale, DMA att rows out as bf16 (host widens
           to f32), PE v2 accumulation (bf16, col-paired into both PSUM
           partition halves)
  phase C: PE sa = (coef*wa) @ v2_aug (+coef*ba via ones row) + x via
           identity matmul, ACT evacuates PSUM, DMA out
"""

import numpy as np

B, C, H, W = 8, 512, 64, 64
N = H * W          # 4096 tokens
S = C // 8         # 64   small channels
CK = C // 128      # 4    c-chunks (partition blocks of x / out)
NT = N // 128      # 32   n-tiles (att row blocks)
MC = N // 512      # 8    m-chunks (512-wide matmul free dim)

_CACHED = None


def _build():
    import concourse.tile as tile
    from concourse import bacc, mybir

    F32 = mybir.dt.float32
    F32R = mybir.dt.float32r
    BF16 = mybir.dt.bfloat16
    EXP = mybir.ActivationFunctionType.Exp
    COPY = mybir.ActivationFunctionType.Copy
    MULT = mybir.AluOpType.mult
    ADD = mybir.AluOpType.add
    AX = mybir.AxisListType.X

    nc = bacc.Bacc("TRN2", target_bir_lowering=False, debug=False)

    x_d = nc.dram_tensor("x", [C, N], F32R, kind="ExternalInput")
    wqT_d = nc.dram_tensor("wqT", [C, S], F32R, kind="ExternalInput")
    wkT_d = nc.dram_tensor("wkT", [C, S], F32R, kind="ExternalInput")
    wvT_d = nc.dram_tensor("wvT", [C, S], F32R, kind="ExternalInput")
    bq_d = nc.dram_tensor("bq", [S, 1], F32, kind="ExternalInput")
    bk_d = nc.dram_tensor("bk", [S, 1], F32, kind="ExternalInput")
    bv_d = nc.dram_tensor("bv", [S, 1], F32, kind="ExternalInput")
    waT_d = nc.dram_tensor("waT", [S, C], F32R, kind="ExternalInput")
    ba_row_d = nc.dram_tensor("ba_row", [1, C], F32R, kind="ExternalInput")
    coef_d = nc.dram_tensor("coefb", [128, 1], F32, kind="ExternalInput")
    id128_d = nc.dram_tensor("ident128", [128, 128], F32R, kind="ExternalInput")
    id64_d = nc.dram_tensor("ident64", [S, S], BF16, kind="ExternalInput")
    ones_d = nc.dram_tensor("ones_row", [1, N], F32R, kind="ExternalInput")

    out_d = nc.dram_tensor("out", [C, N], BF16, kind="ExternalOutput")
    att_d = nc.dram_tensor("att", [N, N], BF16, kind="ExternalOutput")

    with tile.TileContext(nc) as tc:
        with (
            tc.tile_pool(name="persist", bufs=1) as pp,
            tc.tile_pool(name="attp", bufs=3) as attp,
            tc.tile_pool(name="attop", bufs=2) as attop,
            tc.tile_pool(name="outp", bufs=2) as outp,
            tc.tile_pool(name="stats", bufs=8) as statp,
        ):
            # fine-grained tiles so consumers wait only on what they need
            xt = {}
            for j in range(MC):
                for kk in range(CK):
                    xt[(j, kk)] = pp.tile([128, 512], F32R, name=f"x_{j}_{kk}")
            qt = [pp.tile([S, 512], BF16, name=f"q_{j}") for j in range(MC)]
            kt = [pp.tile([S, 512], BF16, name=f"k_{j}") for j in range(MC)]
            vt = [pp.tile([S, 512], BF16, name=f"v_{j}") for j in range(MC)]
            vTt = [pp.tile([128, 4, S], BF16, name=f"vT_{j}") for j in range(MC)]
            v2_aug = pp.tile([S + 1, N], F32R)
            v2h_sb = pp.tile([128, N // 2], F32R)
            wqT_sb = pp.tile([128, CK, S], F32R)
            wkT_sb = pp.tile([128, CK, S], F32R)
            wvT_sb = pp.tile([128, CK, S], F32R)
            waT_aug = pp.tile([S + 1, C], F32R)
            bq_sb = pp.tile([S, 1], F32)
            bk_sb = pp.tile([S, 1], F32)
            bv_sb = pp.tile([S, 1], F32)
            coef_sb = pp.tile([128, 1], F32)
            id128_sb = pp.tile([128, 128], F32R)
            id64_sb = pp.tile([S, S], BF16)

            x_re = x_d.ap().rearrange("(kk p) n -> p kk n", p=128)
            for kk in range(CK):
                nc.sync.dma_start(xt[(0, kk)][:], x_re[:, kk, 0:512])
            for w_sb, w_d in ((wqT_sb, wqT_d), (wkT_sb, wkT_d)):
                nc.sync.dma_start(
                    w_sb[:], w_d.ap().rearrange("(kk p) s -> p kk s", p=128)
                )
            nc.sync.dma_start(bq_sb[:], bq_d.ap())
            nc.sync.dma_start(bk_sb[:], bk_d.ap())
            for j in range(1, MC):
                for kk in range(CK):
                    nc.sync.dma_start(
                        xt[(j, kk)][:], x_re[:, kk, 512 * j : 512 * (j + 1)]
                    )
            nc.sync.dma_start(
                wvT_sb[:], wvT_d.ap().rearrange("(kk p) s -> p kk s", p=128)
            )
            nc.sync.dma_start(bv_sb[:], bv_d.ap())
            nc.sync.dma_start(waT_aug[0:S, :], waT_d.ap())
            nc.sync.dma_start(waT_aug[S : S + 1, :], ba_row_d.ap())
            nc.sync.dma_start(coef_sb[:], coef_d.ap())
            nc.sync.dma_start(id128_sb[:], id128_d.ap())
            nc.sync.dma_start(id64_sb[:], id64_d.ap())

            # sa weights scaled by coef on device; row S carries coef*ba
            nc.vector.tensor_scalar_mul(
                waT_aug[:], waT_aug[:], coef_sb[0 : S + 1, :]
            )
            nc.sync.dma_start(v2_aug[S : S + 1, :], ones_d.ap())

            # one shared pool for qkv psum chunks AND att psum quarters:
            # the PE stream flows from projections straight into attention
            # with no pool barrier in between
            with (
                tc.tile_pool(name="psMain", bufs=2, space="PSUM") as psM,
                tc.tile_pool(name="psV", bufs=1, space="PSUM") as psV,
            ):
                # ---------- phase A: q/k projections (v is folded into
                # the first phase-B iterations)
                def project(w_sb, b_sb, dst, j):
                    ps = psM.tile([S, 512], F32, tag="mm", name="ps_prj")
                    for kk in range(CK):
                        nc.tensor.matmul(
                            ps[:],
                            w_sb[:, kk, :],
                            xt[(j, kk)][:],
                            start=(kk == 0),
                            stop=(kk == CK - 1),
                        )
                    nc.vector.tensor_scalar_add(dst[j][:], ps[:], b_sb[:])

                for j in range(MC):
                    project(wqT_sb, bq_sb, qt, j)
                    project(wkT_sb, bk_sb, kt, j)

                def emit_v_chunk(j):
                    pst = psM.tile([128, 4, S], BF16, tag="mm", name="ps_tp")
                    for t in range(4):
                        nc.tensor.transpose(
                            pst[:, t, :],
                            vt[j][:, 128 * t : 128 * (t + 1)],
                            id64_sb[:],
                        )
                    nc.vector.tensor_copy(vTt[j][:], pst[:])

                # ---------- phase B: attention rows + v2 accumulation
                v2ps = psV.tile([128, 4 * 512], F32)

                def emit_v2(i, asb_i, vTs_i, jlist):
                    # interleave the two psum col-groups so the pairs run
                    # concurrently on the PE quadrants; vTs already carries
                    # the 1/rowsum normalization so the unscaled exp works
                    # as the moving operand
                    for j in jlist:
                        rb = 64 * (j // 4)
                        nc.tensor.matmul(
                            v2ps[rb : rb + 64, 512 * (j % 4) : 512 * (j % 4) + 512],
                            vTs_i[:],
                            asb_i[:, 512 * j : 512 * (j + 1)],
                            start=(i == 0),
                            stop=(i == NT - 1),
                            tile_position=(0, rb),
                            skip_group_check=True,
                        )

                prev = None  # (i, asb) whose v2 matmuls are still pending
                for j in range(MC):
                    project(wvT_sb, bv_sb, vt, j)
                for j in range(MC):
                    emit_v_chunk(j)
                for i in range(NT):
                    asb = attp.tile([128, N], BF16)
                    st = statp.tile([128, 8], F32)
                    kt_i = kt[i // 4]
                    ko = 128 * (i % 4)
                    for qq in range(4):
                        aps = psM.tile([128, 1024], F32, tag="mm")
                        for j in range(2):
                            nc.tensor.matmul(
                                aps[:, 512 * j : 512 * (j + 1)],
                                kt_i[:, ko : ko + 128],
                                qt[2 * qq + j][:],
                                start=True,
                                stop=True,
                            )
                        nc.scalar.activation(
                            asb[:, 1024 * qq : 1024 * (qq + 1)],
                            aps[:],
                            EXP,
                            accum_out=st[:, qq : qq + 1],
                        )
                        if qq == 1 and prev is not None:
                            emit_v2(prev[0], prev[1], prev[2], [0, 4, 1, 5])
                        if qq == 3 and prev is not None:
                            emit_v2(prev[0], prev[1], prev[2], [2, 6])
                    nc.vector.reduce_sum(st[:, 4:5], st[:, 0:4], axis=AX)
                    nc.vector.reciprocal(st[:, 5:6], st[:, 4:5])
                    vTs = statp.tile([128, S], BF16, name="vT_scaled")
                    nc.vector.tensor_scalar_mul(
                        vTs[:], vTt[i // 4][:, i % 4, :], st[:, 5:6]
                    )
                    aob = attop.tile([128, N], BF16)
                    nc.vector.tensor_scalar_mul(aob[:], asb[:], st[:, 5:6])
                    nc.sync.dma_start(att_d.ap()[128 * i : 128 * (i + 1), :], aob[:])
                    if prev is not None:
                        emit_v2(prev[0], prev[1], prev[2], [3, 7])
                    prev = (i, asb, vTs)
                emit_v2(prev[0], prev[1], prev[2], [0, 4, 1, 5, 2, 6, 3, 7])

                # v2ps rows 0-63 hold m-chunks 0-3; rows 64-127 hold 4-7.
                # Everything must land on partitions 0-64 for the sa matmuls:
                # lower half straight via DVE, upper half DVE->SBUF then a
                # cross-partition SBUF->SBUF DMA.
                nc.vector.tensor_copy(v2_aug[0:S, 0 : N // 2], v2ps[0:64, :])
                nc.vector.tensor_copy(v2h_sb[64:128, :], v2ps[64:128, :])
                nc.sync.dma_start(v2_aug[0:S, N // 2 : N], v2h_sb[64:128, :])

            # ---------- phase C: out = coef*(wa@v2 + ba) + x, all on PE
            with tc.tile_pool(name="psD", bufs=2, space="PSUM") as psD:
                for kk in range(CK):
                    for h in range(2):
                        sps = psD.tile([128, 2048], F32)
                        for j in range(4):
                            jj = 4 * h + j
                            nc.tensor.matmul(
                                sps[:, 512 * j : 512 * (j + 1)],
                                waT_aug[:, 128 * kk : 128 * (kk + 1)],
                                v2_aug[:, 512 * jj : 512 * (jj + 1)],
                                start=True,
                                stop=True,
                            )
                        osb = outp.tile([128, 2048], BF16)
                        for j in range(4):
                            jj = 4 * h + j
                            # out = sps (= coef*(wa@v2+ba)) + x
                            nc.vector.scalar_tensor_tensor(
                                osb[:, 512 * j : 512 * (j + 1)],
                                sps[:, 512 * j : 512 * (j + 1)],
                                1.0,
                                xt[(jj, kk)][:].bitcast(F32),
                                op0=MULT,
                                op1=ADD,
                            )
                        nc.sync.dma_start(
                            out_d.ap()[
                                128 * kk : 128 * (kk + 1), 2048 * h : 2048 * (h + 1)
                            ],
                            osb[:],
                        )

    nc.compile()
    return nc


def _get_nc():
    global _CACHED
    if _CACHED is None:
        _CACHED = _build()
    return _CACHED


def make_in_maps(x, wq, bq, wk, bk, wv, bv, wa, ba, coef):
    import ml_dtypes

    x = np.asarray(x, dtype=np.float32)
    xf = np.ascontiguousarray(x.reshape(B, C, N))
    shared = {
        "wqT": np.ascontiguousarray(np.asarray(wq, np.float32).T),
        "wkT": np.ascontiguousarray(np.asarray(wk, np.float32).T),
        "wvT": np.ascontiguousarray(np.asarray(wv, np.float32).T),
        "bq": np.ascontiguousarray(np.asarray(bq, np.float32).reshape(S, 1)),
        "bk": np.ascontiguousarray(np.asarray(bk, np.float32).reshape(S, 1)),
        "bv": np.ascontiguousarray(np.asarray(bv, np.float32).reshape(S, 1)),
        "waT": np.ascontiguousarray(np.asarray(wa, np.float32).T),
        "ba_row": np.ascontiguousarray(np.asarray(ba, np.float32).reshape(1, C)),
        "coefb": np.full((128, 1), np.float32(np.asarray(coef).reshape(-1)[0])),
        "ident128": np.eye(128, dtype=np.float32),
        "ident64": np.eye(S, dtype=ml_dtypes.bfloat16),
        "ones_row": np.ones((1, N), dtype=np.float32),
    }
    return [dict(shared, x=np.ascontiguousarray(xf[b])) for b in range(B)]


def kernel(x, wq, bq, wk, bk, wv, bv, wa, ba, coef, **_unused):
    from concourse.bass_utils import run_bass_kernel_spmd

    nc = _get_nc()
    in_maps = make_in_maps(x, wq, bq, wk, bk, wv, bv, wa, ba, coef)
    res = run_bass_kernel_spmd(nc, in_maps, core_ids=list(range(B)))

    out = np.stack(
        [res.results[b]["out"].astype(np.float32).reshape(C, H, W) for b in range(B)]
    )
    att = np.stack(
        [res.results[b]["att"].astype(np.float32) for b in range(B)]
    )
    return out, att


# revision 20
# speedup vs baseline: 1.1382x; 1.0065x over previous
"""Trainium2 Bass kernel for the sparse-attention module.

Reference computation (per batch element b):
    q = wq @ x + bq ; k = wk @ x + bk ; v = wv @ x + bv        # [S, N]
    att[i, j] = softmax_j( sum_s k[s, i] * q[s, j] )           # [N, N]
    v2 = v @ att                                               # [S, N]
    out = coef * (wa @ v2 + ba) + x                            # [C, N]
returns (out, att).

Sharding: pure data parallel over batch — B=8 batch elements, one per
NeuronCore. Params are tiny and replicated (pre-transposed on host into
PE-friendly layouts).

Per-core dataflow (C=512, N=4096, S=64):
  phase A: stream x in 512-column tiles, project q/k/v on PE (f32r
           matmuls -> bf16), DMA-xbar-transpose v -> vT (bf16).
           qkv PSUM tiles share one pool with phase B's att quarters so
           the PE instruction stream never pauses long enough for the
           HAM clock gate to re-throttle.
  phase B: per 128-row tile of att: PE k^T q (bf16) into double-buffered
           [128,1024] PSUM quarters, ACT exp -> bf16 SBUF, GpSimd row
           sums, DVE 1/sum scale, DMA att rows out as bf16 (host widens
           to f32), PE v2 accumulation (bf16, col-paired into both PSUM
           partition halves)
  phase C: PE sa = (coef*wa) @ v2_aug (+coef*ba via ones row) + x via
           identity matmul, ACT evacuates PSUM, DMA out
"""

import numpy as np

B, C, H, W = 8, 512, 64, 64
N = H * W          # 4096 tokens
S = C // 8         # 64   small channels
CK = C // 128      # 4    c-chunks (partition blocks of x / out)
NT = N // 128      # 32   n-tiles (att row blocks)
MC = N // 512      # 8    m-chunks (512-wide matmul free dim)

_CACHED = None


def _build():
    import concourse.tile as tile
    from concourse import bacc, mybir

    F32 = mybir.dt.float32
    F32R = mybir.dt.float32r
    BF16 = mybir.dt.bfloat16
    EXP = mybir.ActivationFunctionType.Exp
    COPY = mybir.ActivationFunctionType.Copy
    MULT = mybir.AluOpType.mult
    ADD = mybir.AluOpType.add
    AX = mybir.AxisListType.X

    nc = bacc.Bacc("TRN2", target_bir_lowering=False, debug=False)

    x_d = nc.dram_tensor("x", [C, N], F32R, kind="ExternalInput")
    wqT_d = nc.dram_tensor("wqT", [C, S], F32R, kind="ExternalInput")
    wkT_d = nc.dram_tensor("wkT", [C, S], F32R, kind="ExternalInput")
    wvT_d = nc.dram_tensor("wvT", [C, S], F32R, kind="ExternalInput")
    bq_d = nc.dram_tensor("bq", [S, 1], F32, kind="ExternalInput")
    bk_d = nc.dram_tensor("bk", [S, 1], F32, kind="ExternalInput")
    bv_d = nc.dram_tensor("bv", [S, 1], F32, kind="ExternalInput")
    waT_d = nc.dram_tensor("waT", [S, C], F32R, kind="ExternalInput")
    ba_row_d = nc.dram_tensor("ba_row", [1, C], F32R, kind="ExternalInput")
    coef_d = nc.dram_tensor("coefb", [128, 1], F32, kind="ExternalInput")
    id128_d = nc.dram_tensor("ident128", [128, 128], F32R, kind="ExternalInput")
    id64_d = nc.dram_tensor("ident64", [S, S], BF16, kind="ExternalInput")
    ones_d = nc.dram_tensor("ones_row", [1, N], F32R, kind="ExternalInput")

    out_d = nc.dram_tensor("out", [C, N], BF16, kind="ExternalOutput")
    att_d = nc.dram_tensor("att", [N, N], BF16, kind="ExternalOutput")

    with tile.TileContext(nc) as tc:
        with (
            tc.tile_pool(name="persist", bufs=1) as pp,
            tc.tile_pool(name="attp", bufs=3) as attp,
            tc.tile_pool(name="attop", bufs=2) as attop,
            tc.tile_pool(name="outp", bufs=2) as outp,
            tc.tile_pool(name="stats", bufs=8) as statp,
        ):
            # fine-grained tiles so consumers wait only on what they need
            xt = {}
            for j in range(MC):
                xtj = pp.tile([128, CK, 512], F32R, name=f"x_{j}")
                for kk in range(CK):
                    xt[(j, kk)] = xtj[:, kk, :]
                xt[j] = xtj
            qt = [pp.tile([S, 512], BF16, name=f"q_{j}") for j in range(MC)]
            kt = [pp.tile([S, 512], BF16, name=f"k_{j}") for j in range(MC)]
            vt = [pp.tile([S, 512], BF16, name=f"v_{j}") for j in range(MC)]
            vTt = [pp.tile([128, 4, S], BF16, name=f"vT_{j}") for j in range(MC)]
            v2_aug = pp.tile([S + 1, N], F32R)
            v2h_sb = pp.tile([128, N // 2], F32R)
            wqT_sb = pp.tile([128, CK, S], F32R)
            wkT_sb = pp.tile([128, CK, S], F32R)
            wvT_sb = pp.tile([128, CK, S], F32R)
            waT_aug = pp.tile([S + 1, C], F32R)
            bq_sb = pp.tile([S, 1], F32)
            bk_sb = pp.tile([S, 1], F32)
            bv_sb = pp.tile([S, 1], F32)
            coef_sb = pp.tile([128, 1], F32)
            id128_sb = pp.tile([128, 128], F32R)
            id64_sb = pp.tile([S, S], BF16)

            x_re = x_d.ap().rearrange("(kk p) n -> p kk n", p=128)
            nc.sync.dma_start(xt[0][:], x_re[:, :, 0:512])
            for w_sb, w_d in ((wqT_sb, wqT_d), (wkT_sb, wkT_d)):
                nc.sync.dma_start(
                    w_sb[:], w_d.ap().rearrange("(kk p) s -> p kk s", p=128)
                )
            nc.sync.dma_start(bq_sb[:], bq_d.ap())
            nc.sync.dma_start(bk_sb[:], bk_d.ap())
            for j in range(1, MC):
                nc.sync.dma_start(xt[j][:], x_re[:, :, 512 * j : 512 * (j + 1)])
            nc.sync.dma_start(
                wvT_sb[:], wvT_d.ap().rearrange("(kk p) s -> p kk s", p=128)
            )
            nc.sync.dma_start(bv_sb[:], bv_d.ap())
            nc.sync.dma_start(waT_aug[0:S, :], waT_d.ap())
            nc.sync.dma_start(waT_aug[S : S + 1, :], ba_row_d.ap())
            nc.sync.dma_start(coef_sb[:], coef_d.ap())
            nc.sync.dma_start(id128_sb[:], id128_d.ap())
            nc.sync.dma_start(id64_sb[:], id64_d.ap())

            # sa weights scaled by coef on device; row S carries coef*ba
            nc.vector.tensor_scalar_mul(
                waT_aug[:], waT_aug[:], coef_sb[0 : S + 1, :]
            )
            nc.sync.dma_start(v2_aug[S : S + 1, :], ones_d.ap())

            # one shared pool for qkv psum chunks AND att psum quarters:
            # the PE stream flows from projections straight into attention
            # with no pool barrier in between
            with (
                tc.tile_pool(name="psMain", bufs=2, space="PSUM") as psM,
                tc.tile_pool(name="psV", bufs=1, space="PSUM") as psV,
            ):
                # ---------- phase A: q/k projections (v is folded into
                # the first phase-B iterations)
                def project(w_sb, b_sb, dst, j):
                    ps = psM.tile([S, 512], F32, tag="mm", name="ps_prj")
                    for kk in range(CK):
                        nc.tensor.matmul(
                            ps[:],
                            w_sb[:, kk, :],
                            xt[(j, kk)][:],
                            start=(kk == 0),
                            stop=(kk == CK - 1),
                        )
                    nc.vector.tensor_scalar_add(dst[j][:], ps[:], b_sb[:])

                for j in range(MC):
                    project(wqT_sb, bq_sb, qt, j)
                    project(wkT_sb, bk_sb, kt, j)

                def emit_v_chunk(j):
                    pst = psM.tile([128, 4, S], BF16, tag="mm", name="ps_tp")
                    for t in range(4):
                        nc.tensor.transpose(
                            pst[:, t, :],
                            vt[j][:, 128 * t : 128 * (t + 1)],
                            id64_sb[:],
                        )
                    nc.vector.tensor_copy(vTt[j][:], pst[:])

                # ---------- phase B: attention rows + v2 accumulation
                v2ps = psV.tile([128, 4 * 512], F32)

                def emit_v2(i, asb_i, vTs_i, jlist):
                    # interleave the two psum col-groups so the pairs run
                    # concurrently on the PE quadrants; vTs already carries
                    # the 1/rowsum normalization so the unscaled exp works
                    # as the moving operand
                    for j in jlist:
                        rb = 64 * (j // 4)
                        nc.tensor.matmul(
                            v2ps[rb : rb + 64, 512 * (j % 4) : 512 * (j % 4) + 512],
                            vTs_i[:],
                            asb_i[:, 512 * j : 512 * (j + 1)],
                            start=(i == 0),
                            stop=(i == NT - 1),
                            tile_position=(0, rb),
                            skip_group_check=True,
                        )

                prev = None  # (i, asb) whose v2 matmuls are still pending
                for j in range(MC):
                    project(wvT_sb, bv_sb, vt, j)
                for j in range(MC):
                    emit_v_chunk(j)
                for i in range(NT):
                    asb = attp.tile([128, N], BF16)
                    st = statp.tile([128, 8], F32)
                    kt_i = kt[i // 4]
                    ko = 128 * (i % 4)
                    for qq in range(4):
                        aps = psM.tile([128, 1024], F32, tag="mm")
                        for j in range(2):
                            nc.tensor.matmul(
                                aps[:, 512 * j : 512 * (j + 1)],
                                kt_i[:, ko : ko + 128],
                                qt[2 * qq + j][:],
                                start=True,
                                stop=True,
                            )
                        nc.scalar.activation(
                            asb[:, 1024 * qq : 1024 * (qq + 1)],
                            aps[:],
                            EXP,
                            accum_out=st[:, qq : qq + 1],
                        )
                        if qq == 1 and prev is not None:
                            emit_v2(prev[0], prev[1], prev[2], [0, 4, 1, 5])
                        if qq == 3 and prev is not None:
                            emit_v2(prev[0], prev[1], prev[2], [2, 6])
                    nc.vector.reduce_sum(st[:, 4:5], st[:, 0:4], axis=AX)
                    nc.vector.reciprocal(st[:, 5:6], st[:, 4:5])
                    vTs = statp.tile([128, S], BF16, name="vT_scaled")
                    nc.vector.tensor_scalar_mul(
                        vTs[:], vTt[i // 4][:, i % 4, :], st[:, 5:6]
                    )
                    aob = attop.tile([128, N], BF16)
                    nc.vector.tensor_scalar_mul(aob[:], asb[:], st[:, 5:6])
                    nc.sync.dma_start(att_d.ap()[128 * i : 128 * (i + 1), :], aob[:])
                    if prev is not None:
                        emit_v2(prev[0], prev[1], prev[2], [3, 7])
                    prev = (i, asb, vTs)
                emit_v2(prev[0], prev[1], prev[2], [0, 4, 1, 5, 2, 6, 3, 7])

                # v2ps rows 0-63 hold m-chunks 0-3; rows 64-127 hold 4-7.
                # Everything must land on partitions 0-64 for the sa matmuls:
                # lower half straight via DVE, upper half DVE->SBUF then a
                # cross-partition SBUF->SBUF DMA.
                nc.vector.tensor_copy(v2_aug[0:S, 0 : N // 2], v2ps[0:64, :])
                nc.vector.tensor_copy(v2h_sb[64:128, :], v2ps[64:128, :])
                nc.sync.dma_start(v2_aug[0:S, N // 2 : N], v2h_sb[64:128, :])

            # ---------- phase C: out = coef*(wa@v2 + ba) + x, all on PE
            with tc.tile_pool(name="psD", bufs=2, space="PSUM") as psD:
                for kk in range(CK):
                    for h in range(2):
                        sps = psD.tile([128, 2048], F32)
                        for j in range(4):
                            jj = 4 * h + j
                            nc.tensor.matmul(
                                sps[:, 512 * j : 512 * (j + 1)],
                                waT_aug[:, 128 * kk : 128 * (kk + 1)],
                                v2_aug[:, 512 * jj : 512 * (jj + 1)],
                                start=True,
                                stop=True,
                            )
                        osb = outp.tile([128, 2048], BF16)
                        for j in range(4):
                            jj = 4 * h + j
                            # out = sps (= coef*(wa@v2+ba)) + x
                            nc.vector.scalar_tensor_tensor(
                                osb[:, 512 * j : 512 * (j + 1)],
                                sps[:, 512 * j : 512 * (j + 1)],
                                1.0,
                                xt[(jj, kk)][:].bitcast(F32),
                                op0=MULT,
                                op1=ADD,
                            )
                        nc.sync.dma_start(
                            out_d.ap()[
                                128 * kk : 128 * (kk + 1), 2048 * h : 2048 * (h + 1)
                            ],
                            osb[:],
                        )

    nc.compile()
    return nc


def _get_nc():
    global _CACHED
    if _CACHED is None:
        _CACHED = _build()
    return _CACHED


def make_in_maps(x, wq, bq, wk, bk, wv, bv, wa, ba, coef):
    import ml_dtypes

    x = np.asarray(x, dtype=np.float32)
    xf = np.ascontiguousarray(x.reshape(B, C, N))
    shared = {
        "wqT": np.ascontiguousarray(np.asarray(wq, np.float32).T),
        "wkT": np.ascontiguousarray(np.asarray(wk, np.float32).T),
        "wvT": np.ascontiguousarray(np.asarray(wv, np.float32).T),
        "bq": np.ascontiguousarray(np.asarray(bq, np.float32).reshape(S, 1)),
        "bk": np.ascontiguousarray(np.asarray(bk, np.float32).reshape(S, 1)),
        "bv": np.ascontiguousarray(np.asarray(bv, np.float32).reshape(S, 1)),
        "waT": np.ascontiguousarray(np.asarray(wa, np.float32).T),
        "ba_row": np.ascontiguousarray(np.asarray(ba, np.float32).reshape(1, C)),
        "coefb": np.full((128, 1), np.float32(np.asarray(coef).reshape(-1)[0])),
        "ident128": np.eye(128, dtype=np.float32),
        "ident64": np.eye(S, dtype=ml_dtypes.bfloat16),
        "ones_row": np.ones((1, N), dtype=np.float32),
    }
    return [dict(shared, x=np.ascontiguousarray(xf[b])) for b in range(B)]


def kernel(x, wq, bq, wk, bk, wv, bv, wa, ba, coef, **_unused):
    from concourse.bass_utils import run_bass_kernel_spmd

    nc = _get_nc()
    in_maps = make_in_maps(x, wq, bq, wk, bk, wv, bv, wa, ba, coef)
    res = run_bass_kernel_spmd(nc, in_maps, core_ids=list(range(B)))

    out = np.stack(
        [res.results[b]["out"].astype(np.float32).reshape(C, H, W) for b in range(B)]
    )
    att = np.stack(
        [res.results[b]["att"].astype(np.float32) for b in range(B)]
    )
    return out, att


# revision 21
# speedup vs baseline: 1.1973x; 1.0519x over previous
"""Trainium2 Bass kernel for the sparse-attention module.

Reference computation (per batch element b):
    q = wq @ x + bq ; k = wk @ x + bk ; v = wv @ x + bv        # [S, N]
    att[i, j] = softmax_j( sum_s k[s, i] * q[s, j] )           # [N, N]
    v2 = v @ att                                               # [S, N]
    out = coef * (wa @ v2 + ba) + x                            # [C, N]
returns (out, att).

Sharding: pure data parallel over batch — B=8 batch elements, one per
NeuronCore. Params are tiny and replicated (pre-transposed on host into
PE-friendly layouts).

Per-core dataflow (C=512, N=4096, S=64):
  phase A: stream x in 512-column tiles, project q/k/v on PE (f32r
           matmuls -> bf16), DMA-xbar-transpose v -> vT (bf16).
           qkv PSUM tiles share one pool with phase B's att quarters so
           the PE instruction stream never pauses long enough for the
           HAM clock gate to re-throttle.
  phase B: per 128-row tile of att: PE k^T q (bf16) into double-buffered
           [128,1024] PSUM quarters, ACT exp -> bf16 SBUF, GpSimd row
           sums, DVE 1/sum scale, DMA att rows out as bf16 (host widens
           to f32), PE v2 accumulation (bf16, col-paired into both PSUM
           partition halves)
  phase C: PE sa = (coef*wa) @ v2_aug (+coef*ba via ones row) + x via
           identity matmul, ACT evacuates PSUM, DMA out
"""

import numpy as np

B, C, H, W = 8, 512, 64, 64
N = H * W          # 4096 tokens
S = C // 8         # 64   small channels
CK = C // 128      # 4    c-chunks (partition blocks of x / out)
NT = N // 128      # 32   n-tiles (att row blocks)
MC = N // 512      # 8    m-chunks (512-wide matmul free dim)

_CACHED = None


def _build():
    import concourse.tile as tile
    from concourse import bacc, mybir

    F32 = mybir.dt.float32
    F32R = mybir.dt.float32r
    BF16 = mybir.dt.bfloat16
    EXP = mybir.ActivationFunctionType.Exp
    COPY = mybir.ActivationFunctionType.Copy
    MULT = mybir.AluOpType.mult
    ADD = mybir.AluOpType.add
    AX = mybir.AxisListType.X

    nc = bacc.Bacc("TRN2", target_bir_lowering=False, debug=False)

    x_d = nc.dram_tensor("x", [C, N], F32R, kind="ExternalInput")
    wqT_d = nc.dram_tensor("wqT", [C, S], F32R, kind="ExternalInput")
    wkT_d = nc.dram_tensor("wkT", [C, S], F32R, kind="ExternalInput")
    wvT_d = nc.dram_tensor("wvT", [C, S], F32R, kind="ExternalInput")
    bq_d = nc.dram_tensor("bq", [S, 1], F32, kind="ExternalInput")
    bk_d = nc.dram_tensor("bk", [S, 1], F32, kind="ExternalInput")
    bv_d = nc.dram_tensor("bv", [S, 1], F32, kind="ExternalInput")
    waT_d = nc.dram_tensor("waT", [S, C], F32R, kind="ExternalInput")
    ba_row_d = nc.dram_tensor("ba_row", [1, C], F32R, kind="ExternalInput")
    coef_d = nc.dram_tensor("coefb", [128, 1], F32, kind="ExternalInput")
    id128_d = nc.dram_tensor("ident128", [128, 128], F32R, kind="ExternalInput")
    id64_d = nc.dram_tensor("ident64", [S, S], BF16, kind="ExternalInput")
    ones_d = nc.dram_tensor("ones_row", [1, N], F32R, kind="ExternalInput")

    out_d = nc.dram_tensor("out", [C, N], BF16, kind="ExternalOutput")
    att_d = nc.dram_tensor("att", [N, N], BF16, kind="ExternalOutput")

    with tile.TileContext(nc) as tc:
        with (
            tc.tile_pool(name="persist", bufs=1) as pp,
            tc.tile_pool(name="attp", bufs=3) as attp,
            tc.tile_pool(name="attop", bufs=2) as attop,
            tc.tile_pool(name="outp", bufs=2) as outp,
            tc.tile_pool(name="stats", bufs=8) as statp,
        ):
            # fine-grained tiles so consumers wait only on what they need
            xt = {}
            for j in range(MC):
                xtj = pp.tile([128, CK, 512], F32R, name=f"x_{j}")
                for kk in range(CK):
                    xt[(j, kk)] = xtj[:, kk, :]
                xt[j] = xtj
            qt = [pp.tile([S, 512], BF16, name=f"q_{j}") for j in range(MC)]
            kt = [pp.tile([S, 512], BF16, name=f"k_{j}") for j in range(MC)]
            vt = [pp.tile([S, 512], BF16, name=f"v_{j}") for j in range(MC)]
            vTt = [pp.tile([128, 4, S], BF16, name=f"vT_{j}") for j in range(MC)]
            v2_aug = pp.tile([S + 1, N], F32R)
            v2h_sb = pp.tile([128, N // 2], F32R)
            wqT_sb = pp.tile([128, CK, S], F32R)
            wkT_sb = pp.tile([128, CK, S], F32R)
            wvT_sb = pp.tile([128, CK, S], F32R)
            waT_aug = pp.tile([S + 1, C], F32R)
            bq_sb = pp.tile([S, 1], F32)
            bk_sb = pp.tile([S, 1], F32)
            bv_sb = pp.tile([S, 1], F32)
            coef_sb = pp.tile([128, 1], F32)
            id128_sb = pp.tile([128, 128], F32R)
            id64_sb = pp.tile([S, S], BF16)

            x_re = x_d.ap().rearrange("(kk p) n -> p kk n", p=128)
            nc.sync.dma_start(xt[0][:], x_re[:, :, 0:512])
            for w_sb, w_d in ((wqT_sb, wqT_d), (wkT_sb, wkT_d)):
                nc.sync.dma_start(
                    w_sb[:], w_d.ap().rearrange("(kk p) s -> p kk s", p=128)
                )
            nc.sync.dma_start(bq_sb[:], bq_d.ap())
            nc.sync.dma_start(bk_sb[:], bk_d.ap())
            for j in range(1, MC):
                nc.sync.dma_start(xt[j][:], x_re[:, :, 512 * j : 512 * (j + 1)])
            nc.sync.dma_start(
                wvT_sb[:], wvT_d.ap().rearrange("(kk p) s -> p kk s", p=128)
            )
            nc.sync.dma_start(bv_sb[:], bv_d.ap())
            nc.sync.dma_start(waT_aug[0:S, :], waT_d.ap())
            nc.sync.dma_start(waT_aug[S : S + 1, :], ba_row_d.ap())
            nc.sync.dma_start(coef_sb[:], coef_d.ap())
            nc.sync.dma_start(id128_sb[:], id128_d.ap())
            nc.sync.dma_start(id64_sb[:], id64_d.ap())

            # sa weights scaled by coef on device; row S carries coef*ba
            nc.vector.tensor_scalar_mul(
                waT_aug[:], waT_aug[:], coef_sb[0 : S + 1, :]
            )
            nc.sync.dma_start(v2_aug[S : S + 1, :], ones_d.ap())

            # one shared pool for qkv psum chunks AND att psum quarters:
            # the PE stream flows from projections straight into attention
            # with no pool barrier in between
            with (
                tc.tile_pool(name="psMain", bufs=2, space="PSUM") as psM,
                tc.tile_pool(name="psV", bufs=1, space="PSUM") as psV,
            ):
                # ---------- phase A: q/k projections (v is folded into
                # the first phase-B iterations)
                def project(w_sb, b_sb, dst, j):
                    ps = psM.tile([S, 512], F32, tag="mm", name="ps_prj")
                    for kk in range(CK):
                        nc.tensor.matmul(
                            ps[:],
                            w_sb[:, kk, :],
                            xt[(j, kk)][:],
                            start=(kk == 0),
                            stop=(kk == CK - 1),
                        )
                    nc.vector.tensor_scalar_add(dst[j][:], ps[:], b_sb[:])

                for j in range(MC):
                    project(wqT_sb, bq_sb, qt, j)
                    project(wkT_sb, bk_sb, kt, j)

                def emit_v_chunk(j):
                    pst = psM.tile([128, 4, S], BF16, tag="mm", name="ps_tp")
                    for t in range(4):
                        nc.tensor.transpose(
                            pst[:, t, :],
                            vt[j][:, 128 * t : 128 * (t + 1)],
                            id64_sb[:],
                        )
                    nc.vector.tensor_copy(vTt[j][:], pst[:])

                # ---------- phase B: attention rows + v2 accumulation
                v2ps = psV.tile([128, 4 * 512], F32)

                def emit_v2(i, asb_i, vTs_i, jlist):
                    # interleave the two psum col-groups so the pairs run
                    # concurrently on the PE quadrants; vTs already carries
                    # the 1/rowsum normalization so the unscaled exp works
                    # as the moving operand
                    for j in jlist:
                        rb = 64 * (j // 4)
                        nc.tensor.matmul(
                            v2ps[rb : rb + 64, 512 * (j % 4) : 512 * (j % 4) + 512],
                            vTs_i[:],
                            asb_i[:, 512 * j : 512 * (j + 1)],
                            start=(i == 0),
                            stop=(i == NT - 1),
                            tile_position=(0, rb),
                            skip_group_check=True,
                        )

                prev = None  # (i, asb) whose v2 matmuls are still pending
                for j in range(MC):
                    project(wvT_sb, bv_sb, vt, j)
                for j in range(MC):
                    emit_v_chunk(j)
                for i in range(NT):
                    asb = attp.tile([128, N], BF16)
                    st = statp.tile([128, 8], F32)
                    kt_i = kt[i // 4]
                    ko = 128 * (i % 4)
                    for qq in range(4):
                        aps = psM.tile([128, 1024], F32, tag="mm")
                        for j in range(2):
                            nc.tensor.matmul(
                                aps[:, 512 * j : 512 * (j + 1)],
                                kt_i[:, ko : ko + 128],
                                qt[2 * qq + j][:],
                                start=True,
                                stop=True,
                            )
                        if qq < 2:
                            nc.scalar.activation(
                                asb[:, 1024 * qq : 1024 * (qq + 1)],
                                aps[:],
                                EXP,
                            )
                            nc.vector.reduce_sum(
                                st[:, qq : qq + 1],
                                asb[:, 1024 * qq : 1024 * (qq + 1)],
                                axis=AX,
                            )
                        else:
                            nc.scalar.activation(
                                asb[:, 1024 * qq : 1024 * (qq + 1)],
                                aps[:],
                                EXP,
                                accum_out=st[:, qq : qq + 1],
                            )
                        if qq == 1 and prev is not None:
                            emit_v2(prev[0], prev[1], prev[2], [0, 4, 1, 5])
                        if qq == 3 and prev is not None:
                            emit_v2(prev[0], prev[1], prev[2], [2, 6])
                    nc.vector.reduce_sum(st[:, 4:5], st[:, 0:4], axis=AX)
                    nc.vector.reciprocal(st[:, 5:6], st[:, 4:5])
                    vTs = statp.tile([128, S], BF16, name="vT_scaled")
                    nc.vector.tensor_scalar_mul(
                        vTs[:], vTt[i // 4][:, i % 4, :], st[:, 5:6]
                    )
                    aob = attop.tile([128, N], BF16)
                    nc.vector.tensor_scalar_mul(aob[:], asb[:], st[:, 5:6])
                    nc.sync.dma_start(att_d.ap()[128 * i : 128 * (i + 1), :], aob[:])
                    if prev is not None:
                        emit_v2(prev[0], prev[1], prev[2], [3, 7])
                    prev = (i, asb, vTs)
                emit_v2(prev[0], prev[1], prev[2], [0, 4, 1, 5, 2, 6, 3, 7])

                # v2ps rows 0-63 hold m-chunks 0-3; rows 64-127 hold 4-7.
                # Everything must land on partitions 0-64 for the sa matmuls:
                # lower half straight via DVE, upper half DVE->SBUF then a
                # cross-partition SBUF->SBUF DMA.
                nc.vector.tensor_copy(v2_aug[0:S, 0 : N // 2], v2ps[0:64, :])
                nc.vector.tensor_copy(v2h_sb[64:128, :], v2ps[64:128, :])
                nc.sync.dma_start(v2_aug[0:S, N // 2 : N], v2h_sb[64:128, :])

            # ---------- phase C: out = coef*(wa@v2 + ba) + x, all on PE
            with tc.tile_pool(name="psD", bufs=2, space="PSUM") as psD:
                for kk in range(CK):
                    for h in range(2):
                        sps = psD.tile([128, 2048], F32)
                        for j in range(4):
                            jj = 4 * h + j
                            nc.tensor.matmul(
                                sps[:, 512 * j : 512 * (j + 1)],
                                waT_aug[:, 128 * kk : 128 * (kk + 1)],
                                v2_aug[:, 512 * jj : 512 * (jj + 1)],
                                start=True,
                                stop=False,
                                skip_group_check=True,
                            )
                            nc.tensor.matmul(
                                sps[:, 512 * j : 512 * (j + 1)],
                                id128_sb[:],
                                xt[(jj, kk)][:],
                                start=False,
                                stop=True,
                                skip_group_check=True,
                            )
                        osb = outp.tile([128, 2048], BF16)
                        nc.scalar.activation(osb[:], sps[:], COPY)
                        nc.sync.dma_start(
                            out_d.ap()[
                                128 * kk : 128 * (kk + 1), 2048 * h : 2048 * (h + 1)
                            ],
                            osb[:],
                        )

    nc.compile()
    return nc


def _get_nc():
    global _CACHED
    if _CACHED is None:
        _CACHED = _build()
    return _CACHED


def make_in_maps(x, wq, bq, wk, bk, wv, bv, wa, ba, coef):
    import ml_dtypes

    x = np.asarray(x, dtype=np.float32)
    xf = np.ascontiguousarray(x.reshape(B, C, N))
    shared = {
        "wqT": np.ascontiguousarray(np.asarray(wq, np.float32).T),
        "wkT": np.ascontiguousarray(np.asarray(wk, np.float32).T),
        "wvT": np.ascontiguousarray(np.asarray(wv, np.float32).T),
        "bq": np.ascontiguousarray(np.asarray(bq, np.float32).reshape(S, 1)),
        "bk": np.ascontiguousarray(np.asarray(bk, np.float32).reshape(S, 1)),
        "bv": np.ascontiguousarray(np.asarray(bv, np.float32).reshape(S, 1)),
        "waT": np.ascontiguousarray(np.asarray(wa, np.float32).T),
        "ba_row": np.ascontiguousarray(np.asarray(ba, np.float32).reshape(1, C)),
        "coefb": np.full((128, 1), np.float32(np.asarray(coef).reshape(-1)[0])),
        "ident128": np.eye(128, dtype=np.float32),
        "ident64": np.eye(S, dtype=ml_dtypes.bfloat16),
        "ones_row": np.ones((1, N), dtype=np.float32),
    }
    return [dict(shared, x=np.ascontiguousarray(xf[b])) for b in range(B)]


def kernel(x, wq, bq, wk, bk, wv, bv, wa, ba, coef, **_unused):
    from concourse.bass_utils import run_bass_kernel_spmd

    nc = _get_nc()
    in_maps = make_in_maps(x, wq, bq, wk, bk, wv, bv, wa, ba, coef)
    res = run_bass_kernel_spmd(nc, in_maps, core_ids=list(range(B)))

    out = np.stack(
        [res.results[b]["out"].astype(np.float32).reshape(C, H, W) for b in range(B)]
    )
    att = np.stack(
        [res.results[b]["att"].astype(np.float32) for b in range(B)]
    )
    return out, att
